# revision 11
# baseline (speedup 1.0000x reference)
# DenseEdgeConv (B=8, N=4096, D=128, K=16, C=64) Trainium2 Bass kernel, v3.
#
# Data-parallel over B (one point cloud per core). Per core:
#
#   KNN:  ds = -d2 computed on the PE as a 13-row bf16 hi/lo-split matmul
#         (products hi*hi + hi*lo + lo*hi; sq_i/sq_j split hi+lo; error
#         ~2^-17, far below the rank-16/17 distance gap). The DVE then
#         "twiddles" ds: low 12 mantissa bits := column index j (one
#         scalar_tensor_tensor pass, PSUM->SBUF). Ordering keeps 11
#         mantissa bits -- ample for neighbor selection. Top-16 per row
#         via 8 per-512-chunk max8 calls + top-16 of the 64 candidates;
#         neighbor indices fall out of the low bits of the winners (no
#         full-row max_index passes).
#   Gather: indices go to the [16,128]-replicated layout with one XBAR
#         dma transpose, then 4 batched transpose dma_gathers pull the
#         2048 neighbor rows of the bf16 a1-table directly into the
#         transposed [64ch, 2048edge] layout.
#   FC:   algebraic restructure (per-point tables a1/c1..c4):
#           h1 = relu(a1[j] + c1[i] + b1)
#           h2 = relu(W2a^T h1 + c2[i] + b2)
#           h3 = relu(W3a^T h2 + W3b^T h1 + c3[i] + b3)
#           h4 = W4a^T h3 + W4b^T h2 + W4c^T h1   (+ c4[i] + b4 post-max)
#         All matmuls bf16 (1 cycle/row); c-terms broadcast over k with
#         S' = I128 (x) ones(16) selection matmuls. Edge order e = i*16+k
#         (k innermost) so max over k is a contiguous fold, done as bf16
#         tensor-tensor max folds. relu/bias commute with the max
#         (monotone, per-point constant). h1/h2 share one [128, E] tile
#         and h3/h4 another, so each fold covers two layers.
#   out[i] = [h4max + c4 + b4, h3max, h2max, h1max, x[i]]
#
# The per-tile work is software-pipelined: FC lags KNN by 2 tiles and FC
# stages are emitted interleaved with the KNN matmul quarters so no
# engine queue head-blocks.

import numpy as np

import concourse.bacc as bacc
import concourse.bass as bass
import concourse.mybir as mybir
import concourse.tile as tile

FP = mybir.dt.float32
BF = mybir.dt.bfloat16
U32 = mybir.dt.uint32
U16 = mybir.dt.uint16
I16 = mybir.dt.int16

B, N_FULL, D, K, C = 8, 4096, 128, 16, 64
P = 128
E = P * K
AX = mybir.AluOpType
RELU = mybir.ActivationFunctionType.Relu


def _fold(nc, scr, h, out_ap, rows, s0):
    """max over k=16 (innermost, contiguous) of h [rows, (i k)] -> out [rows, 128]."""
    v = h[0:rows, :].rearrange("c (i k) -> c i k", k=16)
    f1 = scr[0:rows, s0:s0 + E // 2].rearrange("c (i k) -> c i k", k=8)
    nc.vector.tensor_tensor(out=f1, in0=v[:, :, 0:8], in1=v[:, :, 8:16], op=AX.max)
    f2 = scr[0:rows, s0 + E // 2:s0 + 3 * E // 4].rearrange("c (i k) -> c i k", k=4)
    nc.vector.tensor_tensor(out=f2, in0=f1[:, :, 0:4], in1=f1[:, :, 4:8], op=AX.max)
    f3 = scr[0:rows, s0 + 3 * E // 4:s0 + 7 * E // 8].rearrange("c (i k) -> c i k", k=2)
    nc.vector.tensor_tensor(out=f3, in0=f2[:, :, 0:2], in1=f2[:, :, 2:4], op=AX.max)
    nc.vector.tensor_tensor(out=out_ap, in0=f3[:, :, 0], in1=f3[:, :, 1], op=AX.max)


def build_kernel(N=N_FULL):
    NT = N // P          # 32 point tiles
    NCH = 8              # knn max8 chunks per row
    CHW = N // NCH       # 512

    nc = bacc.Bacc("TRN2", target_bir_lowering=False, debug=False)

    x_d = nc.dram_tensor("x", [N, D], FP, kind="ExternalInput").ap()
    xt_d = nc.dram_tensor("xt", [D, N], BF, kind="ExternalInput").ap()
    ka_d = nc.dram_tensor("ka", [16, N], BF, kind="ExternalInput").ap()
    kb_d = nc.dram_tensor("kb", [16, N], BF, kind="ExternalInput").ap()
    wcat_d = nc.dram_tensor("wcat", [D, 320], BF, kind="ExternalInput").ap()
    w2a_d = nc.dram_tensor("w2a", [C, C], BF, kind="ExternalInput").ap()
    w3ab_d = nc.dram_tensor("w3ab", [2 * C, C], BF, kind="ExternalInput").ap()
    w4a_d = nc.dram_tensor("w4a", [C, C], BF, kind="ExternalInput").ap()
    w4bc_d = nc.dram_tensor("w4bc", [2 * C, C], BF, kind="ExternalInput").ap()
    b12_d = nc.dram_tensor("b12", [P, 1], FP, kind="ExternalInput").ap()
    b43_d = nc.dram_tensor("b43", [P, 1], FP, kind="ExternalInput").ap()
    b4rb_d = nc.dram_tensor("b4rb", [P, C], BF, kind="ExternalInput").ap()
    sp_d = nc.dram_tensor("sp", [P, E], BF, kind="ExternalInput").ap()
    identfb_d = nc.dram_tensor("identfb", [P, P], BF, kind="ExternalInput").ap()
    identf_d = nc.dram_tensor("identf", [P, P], FP, kind="ExternalInput").ap()
    ibig_d = nc.dram_tensor("ibig", [P, P], FP, kind="ExternalInput").ap()
    iota_d = nc.dram_tensor("iota", [P, N], U32, kind="ExternalInput").ap()

    out_d = nc.dram_tensor("out", [N, D + 4 * C], FP, kind="ExternalOutput").ap()

    with tile.TileContext(nc) as tc:
        with (
            tc.tile_pool(name="const", bufs=1) as cpool,
            tc.tile_pool(name="persist", bufs=1) as ppool,
            tc.tile_pool(name="dram", bufs=1, space="DRAM") as dpool,
        ):
            def cin(name, shape, dt, src):
                tl = cpool.tile(shape, dt, name=name)
                nc.sync.dma_start(tl[:, :], src)
                return tl

            ka = cin("ka", [16, N], BF, ka_d)
            kb = cin("kb", [16, N], BF, kb_d)
            wcat = cin("wcat", [D, 320], BF, wcat_d)
            w2a = cin("w2a", [C, C], BF, w2a_d)
            w3ab = cin("w3ab", [2 * C, C], BF, w3ab_d)
            w4a = cin("w4a", [C, C], BF, w4a_d)
            w4bc = cin("w4bc", [2 * C, C], BF, w4bc_d)
            b12 = cin("b12", [P, 1], FP, b12_d)
            b43 = cin("b43", [P, 1], FP, b43_d)
            b4rb = cin("b4rb", [P, C], BF, b4rb_d)
            sp = cin("sp", [P, E], BF, sp_d)
            identfb = cin("identfb", [P, P], BF, identfb_d)
            identf = cin("identf", [P, P], FP, identf_d)
            ibig = cin("ibig", [P, P], FP, ibig_d)
            iota = cin("iota", [P, N], U32, iota_d)
            msk = cpool.tile([P, 1], U32)
            nc.gpsimd.memset(msk[:, :], int(0xFFFFF000))
            msk12 = cpool.tile([P, 1], U32)
            nc.gpsimd.memset(msk12[:, :], int(0xFFF))

            ctab = ppool.tile([P, NT * 256], BF)   # [c1|c2|c3|c4] per tile
            a1bf = dpool.tile([N, P], BF)          # a1 rows padded to 256B

            # ============ Phase A: per-point tables ============
            with (
                tc.tile_pool(name="axt", bufs=1) as axt,
                tc.tile_pool(name="apsum", bufs=2, space="PSUM") as aps,
                tc.tile_pool(name="asb", bufs=2) as asb,
            ):
                xt = axt.tile([D, N], BF)
                nc.sync.dma_start(xt[:, :], xt_d)
                for t in range(NT):
                    cps = aps.tile([P, 320], FP, tag="cps")
                    nc.tensor.matmul(cps[:, :], lhsT=xt[:, t * P:(t + 1) * P],
                                     rhs=wcat[:, :], start=True, stop=True)
                    a1s = asb.tile([P, P], BF, tag="a1s")
                    nc.gpsimd.memset(a1s[:, C:P], 0.0)
                    nc.scalar.copy(a1s[:, 0:C], cps[:, 0:C])
                    nc.sync.dma_start(a1bf[t * P:(t + 1) * P, :], a1s[:, :])
                    nc.scalar.copy(ctab[:, t * 256:(t + 1) * 256], cps[:, C:320])
                    nc.vector.tensor_tensor(
                        out=ctab[:, t * 256 + 192:t * 256 + 256],
                        in0=ctab[:, t * 256 + 192:t * 256 + 256],
                        in1=b4rb[:, :], op=AX.add)

            # ============ Phase B: fused KNN + FC, software-pipelined ======
            with (
                tc.tile_pool(name="kps", bufs=2, space="PSUM") as kps,
                tc.tile_pool(name="fps", bufs=1, space="PSUM") as fps,
                tc.tile_pool(name="knnsb", bufs=1) as ksb,
                tc.tile_pool(name="ring", bufs=2) as ring,
            ):
                t12 = ksb.tile([P, 16], FP)
                ix = ksb.tile([P, 16], U32)
                ds_ring = {}
                cand_ring = {}

                def knn_mm_stage(t, q):
                    """One PSUM quarter of distances, twiddle, chunk maxes."""
                    if q == 0:
                        ds_ring[t] = ring.tile([P, N], FP, tag="ds",
                                               name=f"ds_{t}")
                        cand_ring[t] = ring.tile([P, NCH * 8], FP, tag="cand",
                                                 name=f"cand_{t}")
                    ds = ds_ring[t]
                    cand = cand_ring[t]
                    dq = kps.tile([P, 1024], FP, tag="knn", name=f"dq_{t}_{q}")
                    for h in range(2):
                        c0 = h * 512
                        nc.tensor.matmul(dq[:, c0:c0 + 512],
                                         lhsT=ka[:, t * P:(t + 1) * P],
                                         rhs=kb[:, q * 1024 + c0:q * 1024 + c0 + 512],
                                         start=True, stop=True)
                    nc.vector.scalar_tensor_tensor(
                        out=ds[:, q * 1024:(q + 1) * 1024].bitcast(U32),
                        in0=dq[:, :].bitcast(U32), scalar=msk[:, :],
                        in1=iota[:, q * 1024:(q + 1) * 1024],
                        op0=AX.bitwise_and, op1=AX.bitwise_or)
                    if q == t // 8:
                        # self-distance kill lives in this quarter
                        nc.vector.tensor_tensor(out=ds[:, t * P:(t + 1) * P],
                                                in0=ds[:, t * P:(t + 1) * P],
                                                in1=ibig[:, :], op=AX.subtract)
                    for ch in (2 * q, 2 * q + 1):
                        nc.vector.max(cand[:, ch * 8:(ch + 1) * 8],
                                      ds[:, ch * CHW:(ch + 1) * CHW])

                def knn_select(s):
                    """Top-16 of the chunk candidates, index extract, gather."""
                    ds_ring.pop(s)
                    cand = cand_ring.pop(s)
                    nc.vector.max(t12[:, 0:8], cand[:, :])
                    nc.vector.match_replace(out=cand[:, :],
                                            in_to_replace=t12[:, 0:8],
                                            in_values=cand[:, :],
                                            imm_value=-1.0e30)
                    nc.vector.max(t12[:, 8:16], cand[:, :])
                    nc.vector.tensor_scalar(out=ix[:, :],
                                            in0=t12[:, :].bitcast(U32),
                                            scalar1=msk12[:, :], scalar2=None,
                                            op0=AX.bitwise_and)
                    idxJ = ring.tile([P, P], I16, tag="idxJ")
                    nc.vector.tensor_copy(
                        idxJ[:, 0:16].bitcast(U16),
                        ix[:, :].bitcast(U16)
                        .rearrange("p (k two) -> p k two", two=2)[:, :, 0])
                    nc.vector.tensor_copy(idxJ[:, 16:32], idxJ[:, 0:16])
                    nc.vector.tensor_copy(idxJ[:, 32:64], idxJ[:, 0:32])
                    nc.vector.tensor_copy(idxJ[:, 64:128], idxJ[:, 0:64])
                    idxT = ring.tile([P, P], I16, tag="idxT")
                    nc.sync.dma_start_transpose(idxT[:, :], idxJ[:, :])
                    a1g = ring.tile([P, K * P], BF, tag="a1g", bufs=3)
                    a1gv = a1g[:, :].rearrange("p (b c) -> p b c", b=K)
                    for g in range(2):
                        nc.gpsimd.dma_gather(
                            out_ap=a1gv[:, g * 8:(g + 1) * 8, :],
                            in_ap=a1bf[:, :],
                            idxs_ap=idxT[:, g * 64:(g + 1) * 64],
                            num_idxs=1024, num_idxs_reg=1024, elem_size=P,
                            transpose=False)
                    return a1g

                def fc_stages(u, a1g):
                    """Generator: one FC tile in 5 stages (yield between them)."""
                    co = u * 256
                    hstack = ring.tile([P, E], BF, tag="hstack")
                    h34 = ring.tile([P, E], BF, tag="h34")
                    scr = ring.tile([P, 2 * E], BF, tag="scr")
                    msbA = ring.tile([P, P], FP, tag="msbA")
                    msbB = ring.tile([P, P], FP, tag="msbB")
                    psf = fps.tile([P, E], FP, tag="fc", name=f"psf_{u}")

                    # --- stage 1: layer 1 -> psf[0:64]
                    a1gv = a1g[:, :].rearrange("p (b c) -> p b c", b=K)
                    for bb in range(K):
                        # start=True resets the whole PSUM bank: only the
                        # first of the 4 sub-bank block matmuls may set it
                        nc.tensor.matmul(psf[0:C, bb * P:(bb + 1) * P],
                                         lhsT=a1gv[:, bb, 0:C],
                                         rhs=identfb[:, :],
                                         start=(bb % 4 == 0), stop=False)
                    for n in range(4):
                        nc.tensor.matmul(psf[0:C, n * 512:(n + 1) * 512],
                                         lhsT=ctab[:, co:co + C],
                                         rhs=sp[:, n * 512:(n + 1) * 512],
                                         start=False, stop=True)
                    for hh in range(2):
                        nc.scalar.activation(
                            hstack[0:C, hh * 1024:(hh + 1) * 1024],
                            psf[0:C, hh * 1024:(hh + 1) * 1024], RELU,
                            bias=b12[0:C, :], scale=1.0)
                    yield
                    # --- stage 2: layer 2 -> psf[64:128]
                    for n in range(4):
                        nc.tensor.matmul(psf[C:P, n * 512:(n + 1) * 512],
                                         lhsT=w2a[:, :],
                                         rhs=hstack[0:C, n * 512:(n + 1) * 512],
                                         start=True, stop=False)
                    for n in range(4):
                        nc.tensor.matmul(psf[C:P, n * 512:(n + 1) * 512],
                                         lhsT=ctab[:, co + C:co + 2 * C],
                                         rhs=sp[:, n * 512:(n + 1) * 512],
                                         start=False, stop=True)
                    for hh in range(2):
                        nc.scalar.activation(
                            hstack[C:P, hh * 1024:(hh + 1) * 1024],
                            psf[C:P, hh * 1024:(hh + 1) * 1024], RELU,
                            bias=b12[C:P, :], scale=1.0)
                    yield
                    # --- stage 3: layer 3 -> psf[0:64]; h1|h2 fold
                    for n in range(4):
                        nc.tensor.matmul(psf[0:C, n * 512:(n + 1) * 512],
                                         lhsT=w3ab[:, :],
                                         rhs=hstack[:, n * 512:(n + 1) * 512],
                                         start=True, stop=False)
                    for n in range(4):
                        nc.tensor.matmul(psf[0:C, n * 512:(n + 1) * 512],
                                         lhsT=ctab[:, co + 2 * C:co + 3 * C],
                                         rhs=sp[:, n * 512:(n + 1) * 512],
                                         start=False, stop=True)
                    _fold(nc, scr, hstack, msbB[:, :], P, 0)
                    for hh in range(2):
                        nc.scalar.activation(
                            h34[0:C, hh * 1024:(hh + 1) * 1024],
                            psf[0:C, hh * 1024:(hh + 1) * 1024], RELU,
                            bias=b43[0:C, :], scale=1.0)
                    yield
                    # --- stage 4: layer 4 -> psf[64:128]
                    for n in range(4):
                        nc.tensor.matmul(psf[C:P, n * 512:(n + 1) * 512],
                                         lhsT=w4a[:, :],
                                         rhs=h34[0:C, n * 512:(n + 1) * 512],
                                         start=True, stop=False)
                    for n in range(4):
                        nc.tensor.matmul(psf[C:P, n * 512:(n + 1) * 512],
                                         lhsT=w4bc[:, :],
                                         rhs=hstack[:, n * 512:(n + 1) * 512],
                                         start=False, stop=True)
                    for hh in range(2):
                        nc.scalar.copy(h34[C:P, hh * 1024:(hh + 1) * 1024],
                                       psf[C:P, hh * 1024:(hh + 1) * 1024])
                    yield
                    # --- stage 5: h3|h4 fold, post-max relu, transpose, out
                    _fold(nc, scr, h34, msbA[:, :], P, E)
                    nc.scalar.activation(msbB[:, :], msbB[:, :], RELU,
                                         bias=b12[:, :], scale=1.0)
                    nc.scalar.activation(msbA[0:C, :], msbA[0:C, :], RELU,
                                         bias=b43[0:C, :], scale=1.0)
                    pso = fps.tile([P, 2 * P], FP, tag="fc", name=f"pso_{u}")
                    nc.tensor.matmul(pso[:, 0:P], lhsT=msbA[:, :],
                                     rhs=identf[:, :], is_transpose=True,
                                     start=True, stop=False)
                    nc.tensor.matmul(pso[:, P:2 * P], lhsT=msbB[:, :],
                                     rhs=identf[:, :], is_transpose=True,
                                     start=False, stop=True)
                    outsb = ring.tile([P, D + 4 * C], FP, tag="outsb")
                    # order: [h4, h3, h2, h1, x]; msbA=[h3;h4], msbB=[h1;h2]
                    nc.scalar.copy(outsb[:, 0:C], pso[:, C:2 * C])
                    nc.scalar.copy(outsb[:, C:2 * C], pso[:, 0:C])
                    nc.scalar.copy(outsb[:, 2 * C:3 * C], pso[:, 3 * C:4 * C])
                    nc.scalar.copy(outsb[:, 3 * C:4 * C], pso[:, 2 * C:3 * C])
                    nc.vector.tensor_tensor(
                        out=outsb[:, 0:C], in0=outsb[:, 0:C],
                        in1=ctab[:, co + 3 * C:co + 4 * C], op=AX.add)
                    nc.sync.dma_start(outsb[:, 4 * C:4 * C + D],
                                      x_d[u * P:(u + 1) * P, :])
                    nc.sync.dma_start(out_d[u * P:(u + 1) * P, :], outsb[:, :])
                    yield

                def adv(g):
                    if g is not None:
                        next(g, None)

                # KNN mms at t, selection+gather at t-1, FC at t-3.
                a1g_ring = {}
                for it in range(NT + 3):
                    t, s, u = it, it - 1, it - 3
                    g = fc_stages(u, a1g_ring.pop(u)) if 0 <= u < NT else None
                    if 0 <= s < NT:
                        a1g_ring[s] = knn_select(s)
                    adv(g)                      # FC(u) stage 1
                    if t < NT:
                        knn_mm_stage(t, 0)
                    adv(g)                      # FC(u) stage 2
                    if t < NT:
                        knn_mm_stage(t, 1)
                    adv(g)                      # FC(u) stage 3
                    if t < NT:
                        knn_mm_stage(t, 2)
                    adv(g)                      # FC(u) stage 4
                    if t < NT:
                        knn_mm_stage(t, 3)
                    adv(g)                      # FC(u) stage 5

    nc.compile()
    return nc


def host_prep(x, pos, W_first, b_first, W_mid1, b_mid1, W_mid2, b_mid2,
              W_last, b_last):
    """Host-side arrangement of per-core inputs (numpy, cheap O(N) work)."""
    import ml_dtypes
    f32 = np.float32
    bf = ml_dtypes.bfloat16
    x = np.asarray(x, f32)
    pos = np.asarray(pos, f32)
    Wf = np.asarray(W_first, f32)
    Wm1 = np.asarray(W_mid1, f32)
    Wm2 = np.asarray(W_mid2, f32)
    Wl = np.asarray(W_last, f32)

    V1 = Wf[D:2 * D] + Wf[2 * D:3 * D]
    U1 = Wf[0:D] - Wf[2 * D:3 * D]
    W2a, W2x = Wm1[0:C], Wm1[C:C + D]
    W3a, W3b, W3c = Wm2[0:C], Wm2[C:2 * C], Wm2[2 * C:2 * C + D]
    W4a, W4b, W4c, W4d = Wl[0:C], Wl[C:2 * C], Wl[2 * C:3 * C], Wl[3 * C:3 * C + D]

    n = x.shape[1]
    b1 = np.asarray(b_first, f32).reshape(C, 1)
    b2 = np.asarray(b_mid1, f32).reshape(C, 1)
    b3v = np.asarray(b_mid2, f32).reshape(C, 1)

    e_idx = np.arange(P * K)
    sp = (np.arange(P)[:, None] == (e_idx // K)[None, :]).astype(bf)

    shared = {
        "wcat": np.concatenate([V1, U1, W2x, W3c, W4d], axis=1).astype(bf),
        "w2a": W2a.astype(bf),
        # hstack rows: [h1 (0:64); h2 (64:128)]
        "w3ab": np.concatenate([W3b, W3a], axis=0).astype(bf),
        "w4a": W4a.astype(bf),
        "w4bc": np.concatenate([W4c, W4b], axis=0).astype(bf),
        "b12": np.concatenate([b1, b2], axis=0).copy(),
        "b43": np.concatenate([b3v, np.zeros_like(b3v)], axis=0).copy(),
        "b4rb": np.broadcast_to(np.asarray(b_last, f32).reshape(1, C),
                                (P, C)).astype(bf),
        "sp": np.ascontiguousarray(sp),
        "identfb": np.eye(P, dtype=f32).astype(bf),
        "identf": np.eye(P, dtype=f32),
        "ibig": (np.eye(P, dtype=f32) * 1.0e38),
        "iota": np.broadcast_to(np.arange(n, dtype=np.uint32), (P, n)).copy(),
    }

    in_maps = []
    for bi in range(x.shape[0]):
        pb = pos[bi]                                  # (N, 3)
        sq = (pb * pb).sum(axis=-1, dtype=f32)        # (N,)
        ph = pb.astype(bf)
        pl = (pb - ph.astype(f32)).astype(bf)
        sqh = sq.astype(bf)
        sql = (sq - sqh.astype(f32)).astype(bf)
        ones = np.ones(n, f32)

        ka = np.zeros((16, n), f32)
        kb = np.zeros((16, n), f32)
        ph32, pl32 = ph.astype(f32), pl.astype(f32)
        ka[0:3] = 2.0 * ph32.T
        kb[0:3] = ph32.T
        ka[3:6] = 2.0 * ph32.T
        kb[3:6] = pl32.T
        ka[6:9] = 2.0 * pl32.T
        kb[6:9] = ph32.T
        ka[9] = sqh.astype(f32)
        kb[9] = -ones
        ka[10] = sql.astype(f32)
        kb[10] = -ones
        ka[11] = -ones
        kb[11] = sqh.astype(f32)
        ka[12] = -ones
        kb[12] = sql.astype(f32)

        m = dict(shared)
        m["x"] = np.ascontiguousarray(x[bi])
        m["xt"] = np.ascontiguousarray(x[bi].T).astype(bf)
        m["ka"] = ka.astype(bf)
        m["kb"] = kb.astype(bf)
        in_maps.append(m)
    return in_maps


_NC_CACHE = {}
LAST_RESULT = None


def kernel(**inputs):
    import os

    from concourse.bass_utils import run_bass_kernel_spmd

    global LAST_RESULT
    in_maps = host_prep(**inputs)
    n = inputs["x"].shape[1]
    if n not in _NC_CACHE:
        _NC_CACHE[n] = build_kernel(n)
    nc = _NC_CACHE[n]
    trace = bool(os.environ.get("KERNEL_TRACE"))
    res = run_bass_kernel_spmd(nc, in_maps, core_ids=list(range(len(in_maps))),
                               trace=trace)
    LAST_RESULT = res
    out = np.stack([r["out"] for r in res.results], axis=0)
    return out


# revision 13
# speedup vs baseline: 1.2658x; 1.2658x over previous
# DenseEdgeConv (B=8, N=4096, D=128, K=16, C=64) Trainium2 Bass kernel, v3.
#
# Data-parallel over B (one point cloud per core). Per core:
#
#   KNN:  ds = -d2 computed on the PE as a 13-row bf16 hi/lo-split matmul
#         (products hi*hi + hi*lo + lo*hi; sq_i/sq_j split hi+lo; error
#         ~2^-17, far below the rank-16/17 distance gap). The DVE then
#         "twiddles" ds: low 12 mantissa bits := column index j (one
#         scalar_tensor_tensor pass, PSUM->SBUF). Ordering keeps 11
#         mantissa bits -- ample for neighbor selection. Top-16 per row
#         via 8 per-512-chunk max8 calls + top-16 of the 64 candidates;
#         neighbor indices fall out of the low bits of the winners (no
#         full-row max_index passes).
#   Gather: indices go to the [16,128]-replicated layout with one XBAR
#         dma transpose, then 4 batched transpose dma_gathers pull the
#         2048 neighbor rows of the bf16 a1-table directly into the
#         transposed [64ch, 2048edge] layout.
#   FC:   algebraic restructure (per-point tables a1/c1..c4):
#           h1 = relu(a1[j] + c1[i] + b1)
#           h2 = relu(W2a^T h1 + c2[i] + b2)
#           h3 = relu(W3a^T h2 + W3b^T h1 + c3[i] + b3)
#           h4 = W4a^T h3 + W4b^T h2 + W4c^T h1   (+ c4[i] + b4 post-max)
#         All matmuls bf16 (1 cycle/row); c-terms broadcast over k with
#         S' = I128 (x) ones(16) selection matmuls. Edge order e = i*16+k
#         (k innermost) so max over k is a contiguous fold, done as bf16
#         tensor-tensor max folds. relu/bias commute with the max
#         (monotone, per-point constant). h1/h2 share one [128, E] tile
#         and h3/h4 another, so each fold covers two layers.
#   out[i] = [h4max + c4 + b4, h3max, h2max, h1max, x[i]]
#
# The per-tile work is software-pipelined: FC lags KNN by 2 tiles and FC
# stages are emitted interleaved with the KNN matmul quarters so no
# engine queue head-blocks.

import numpy as np

import concourse.bacc as bacc
import concourse.bass as bass
import concourse.mybir as mybir
import concourse.tile as tile

FP = mybir.dt.float32
BF = mybir.dt.bfloat16
U32 = mybir.dt.uint32
U16 = mybir.dt.uint16
I16 = mybir.dt.int16

B, N_FULL, D, K, C = 8, 4096, 128, 16, 64
P = 128
E = P * K
AX = mybir.AluOpType
RELU = mybir.ActivationFunctionType.Relu


def _fold(nc, scr, h, out_ap, rows, s0):
    """max over k=16 (innermost, contiguous) of h [rows, (i k)] -> out [rows, 128]."""
    v = h[0:rows, :].rearrange("c (i k) -> c i k", k=16)
    f1 = scr[0:rows, s0:s0 + E // 2].rearrange("c (i k) -> c i k", k=8)
    nc.vector.tensor_tensor(out=f1, in0=v[:, :, 0:8], in1=v[:, :, 8:16], op=AX.max)
    f2 = scr[0:rows, s0 + E // 2:s0 + 3 * E // 4].rearrange("c (i k) -> c i k", k=4)
    nc.vector.tensor_tensor(out=f2, in0=f1[:, :, 0:4], in1=f1[:, :, 4:8], op=AX.max)
    f3 = scr[0:rows, s0 + 3 * E // 4:s0 + 7 * E // 8].rearrange("c (i k) -> c i k", k=2)
    nc.vector.tensor_tensor(out=f3, in0=f2[:, :, 0:2], in1=f2[:, :, 2:4], op=AX.max)
    nc.vector.tensor_tensor(out=out_ap, in0=f3[:, :, 0], in1=f3[:, :, 1], op=AX.max)


def build_kernel(N=N_FULL):
    NT = N // P          # 32 point tiles
    NCH = 8              # knn max8 chunks per row
    CHW = N // NCH       # 512

    nc = bacc.Bacc("TRN2", target_bir_lowering=False, debug=False)

    x_d = nc.dram_tensor("x", [N, D], FP, kind="ExternalInput").ap()
    xt_d = nc.dram_tensor("xt", [D, N], BF, kind="ExternalInput").ap()
    ka_d = nc.dram_tensor("ka", [16, N], BF, kind="ExternalInput").ap()
    kb_d = nc.dram_tensor("kb", [16, N], BF, kind="ExternalInput").ap()
    wcat_d = nc.dram_tensor("wcat", [D, 320], BF, kind="ExternalInput").ap()
    w2a_d = nc.dram_tensor("w2a", [C, C], BF, kind="ExternalInput").ap()
    w3ab_d = nc.dram_tensor("w3ab", [2 * C, C], BF, kind="ExternalInput").ap()
    w4a_d = nc.dram_tensor("w4a", [C, C], BF, kind="ExternalInput").ap()
    w4bc_d = nc.dram_tensor("w4bc", [2 * C, C], BF, kind="ExternalInput").ap()
    b12_d = nc.dram_tensor("b12", [P, 1], FP, kind="ExternalInput").ap()
    b43_d = nc.dram_tensor("b43", [P, 1], FP, kind="ExternalInput").ap()
    b4rb_d = nc.dram_tensor("b4rb", [P, C], BF, kind="ExternalInput").ap()
    sp_d = nc.dram_tensor("sp", [P, E], BF, kind="ExternalInput").ap()
    identfb_d = nc.dram_tensor("identfb", [P, P], BF, kind="ExternalInput").ap()
    identf_d = nc.dram_tensor("identf", [P, P], FP, kind="ExternalInput").ap()
    ibig_d = nc.dram_tensor("ibig", [P, P], FP, kind="ExternalInput").ap()
    iota_d = nc.dram_tensor("iota", [P, N], U32, kind="ExternalInput").ap()

    out_d = nc.dram_tensor("out", [N, D + 4 * C], FP, kind="ExternalOutput").ap()

    with tile.TileContext(nc) as tc:
        with (
            tc.tile_pool(name="const", bufs=1) as cpool,
            tc.tile_pool(name="persist", bufs=1) as ppool,
            tc.tile_pool(name="dram", bufs=1, space="DRAM") as dpool,
        ):
            def cin(name, shape, dt, src):
                tl = cpool.tile(shape, dt, name=name)
                nc.sync.dma_start(tl[:, :], src)
                return tl

            ka = cin("ka", [16, N], BF, ka_d)
            kb = cin("kb", [16, N], BF, kb_d)
            wcat = cin("wcat", [D, 320], BF, wcat_d)
            w2a = cin("w2a", [C, C], BF, w2a_d)
            w3ab = cin("w3ab", [2 * C, C], BF, w3ab_d)
            w4a = cin("w4a", [C, C], BF, w4a_d)
            w4bc = cin("w4bc", [2 * C, C], BF, w4bc_d)
            b12 = cin("b12", [P, 1], FP, b12_d)
            b43 = cin("b43", [P, 1], FP, b43_d)
            b4rb = cin("b4rb", [P, C], BF, b4rb_d)
            sp = cin("sp", [P, E], BF, sp_d)
            identfb = cin("identfb", [P, P], BF, identfb_d)
            identf = cin("identf", [P, P], FP, identf_d)
            ibig = cin("ibig", [P, P], FP, ibig_d)
            iota = cin("iota", [P, N], U32, iota_d)
            msk = cpool.tile([P, 1], U32)
            nc.gpsimd.memset(msk[:, :], int(0xFFFFF000))
            msk12 = cpool.tile([P, 1], U32)
            nc.gpsimd.memset(msk12[:, :], int(0xFFF))

            ctab = ppool.tile([P, NT * 256], BF)   # [c1|c2|c3|c4] per tile
            a1bf = dpool.tile([N, P], BF)          # a1 rows padded to 256B

            # ============ Phase A: per-point tables ============
            with (
                tc.tile_pool(name="axt", bufs=1) as axt,
                tc.tile_pool(name="apsum", bufs=2, space="PSUM") as aps,
                tc.tile_pool(name="asb", bufs=2) as asb,
            ):
                xt = axt.tile([D, N], BF)
                nc.sync.dma_start(xt[:, :], xt_d)
                for t in range(NT):
                    cps = aps.tile([P, 320], FP, tag="cps")
                    nc.tensor.matmul(cps[:, :], lhsT=xt[:, t * P:(t + 1) * P],
                                     rhs=wcat[:, :], start=True, stop=True)
                    a1s = asb.tile([P, P], BF, tag="a1s")
                    nc.gpsimd.memset(a1s[:, C:P], 0.0)
                    nc.scalar.copy(a1s[:, 0:C], cps[:, 0:C])
                    nc.sync.dma_start(a1bf[t * P:(t + 1) * P, :], a1s[:, :])
                    nc.scalar.copy(ctab[:, t * 256:(t + 1) * 256], cps[:, C:320])
                    nc.vector.tensor_tensor(
                        out=ctab[:, t * 256 + 192:t * 256 + 256],
                        in0=ctab[:, t * 256 + 192:t * 256 + 256],
                        in1=b4rb[:, :], op=AX.add)

            # ============ Phase B: fused KNN + FC, software-pipelined ======
            with (
                tc.tile_pool(name="kps", bufs=1, space="PSUM") as kps,
                tc.tile_pool(name="pps", bufs=2, space="PSUM") as pps,
                tc.tile_pool(name="fps", bufs=1, space="PSUM") as fps,
                tc.tile_pool(name="knnsb", bufs=1) as ksb,
                tc.tile_pool(name="ring", bufs=2) as ring,
            ):
                t12 = ksb.tile([P, 16], FP)
                ix = ksb.tile([P, 16], U32)
                ds_ring = {}
                cand_ring = {}

                def knn_mm_stage(t, q):
                    """One PSUM quarter of distances, twiddle, chunk maxes."""
                    if q == 0:
                        ds_ring[t] = ring.tile([P, N], FP, tag="ds",
                                               name=f"ds_{t}")
                        cand_ring[t] = ring.tile([P, NCH * 8], FP, tag="cand",
                                                 name=f"cand_{t}")
                    ds = ds_ring[t]
                    cand = cand_ring[t]
                    dq = kps.tile([P, 1024], FP, tag="knn", name=f"dq_{t}_{q}")
                    for h in range(2):
                        c0 = h * 512
                        nc.tensor.matmul(dq[:, c0:c0 + 512],
                                         lhsT=ka[:, t * P:(t + 1) * P],
                                         rhs=kb[:, q * 1024 + c0:q * 1024 + c0 + 512],
                                         start=True, stop=True)
                    nc.vector.scalar_tensor_tensor(
                        out=ds[:, q * 1024:(q + 1) * 1024].bitcast(U32),
                        in0=dq[:, :].bitcast(U32), scalar=msk[:, :],
                        in1=iota[:, q * 1024:(q + 1) * 1024],
                        op0=AX.bitwise_and, op1=AX.bitwise_or)
                    if q == t // 8:
                        # self-distance kill lives in this quarter
                        nc.vector.tensor_tensor(out=ds[:, t * P:(t + 1) * P],
                                                in0=ds[:, t * P:(t + 1) * P],
                                                in1=ibig[:, :], op=AX.subtract)
                    for ch in (2 * q, 2 * q + 1):
                        nc.vector.max(cand[:, ch * 8:(ch + 1) * 8],
                                      ds[:, ch * CHW:(ch + 1) * CHW])

                def knn_select(s):
                    """Top-16 of the chunk candidates, index extract, gather."""
                    ds_ring.pop(s)
                    cand = cand_ring.pop(s)
                    nc.vector.max(t12[:, 0:8], cand[:, :])
                    nc.vector.match_replace(out=cand[:, :],
                                            in_to_replace=t12[:, 0:8],
                                            in_values=cand[:, :],
                                            imm_value=-1.0e30)
                    nc.vector.max(t12[:, 8:16], cand[:, :])
                    nc.vector.tensor_scalar(out=ix[:, :],
                                            in0=t12[:, :].bitcast(U32),
                                            scalar1=msk12[:, :], scalar2=None,
                                            op0=AX.bitwise_and)
                    idxJ = ring.tile([P, P], I16, tag="idxJ")
                    nc.vector.tensor_copy(
                        idxJ[:, 0:16].bitcast(U16),
                        ix[:, :].bitcast(U16)
                        .rearrange("p (k two) -> p k two", two=2)[:, :, 0])
                    nc.vector.tensor_copy(idxJ[:, 16:32], idxJ[:, 0:16])
                    nc.vector.tensor_copy(idxJ[:, 32:64], idxJ[:, 0:32])
                    nc.vector.tensor_copy(idxJ[:, 64:128], idxJ[:, 0:64])
                    idxT = ring.tile([P, P], I16, tag="idxT")
                    nc.sync.dma_start_transpose(idxT[:, :], idxJ[:, :])
                    a1g = ring.tile([P, K * P], BF, tag="a1g", bufs=3)
                    a1gv = a1g[:, :].rearrange("p (b c) -> p b c", b=K)
                    for g in range(2):
                        nc.gpsimd.dma_gather(
                            out_ap=a1gv[:, g * 8:(g + 1) * 8, :],
                            in_ap=a1bf[:, :],
                            idxs_ap=idxT[:, g * 64:(g + 1) * 64],
                            num_idxs=1024, num_idxs_reg=1024, elem_size=P,
                            transpose=False)
                    return a1g

                def fc_stages(u, a1g):
                    """Generator: one FC tile in 5 stages (yield between them)."""
                    co = u * 256
                    hstack = ring.tile([P, E], BF, tag="hstack")
                    h34 = ring.tile([P, E], BF, tag="h34")
                    scr = ring.tile([P, 2 * E], BF, tag="scr")
                    msbA = ring.tile([P, P], FP, tag="msbA")
                    msbB = ring.tile([P, P], FP, tag="msbB")
                    psf = fps.tile([P, E], FP, tag="fc", name=f"psf_{u}")

                    # --- stage 1: layer 1 -> psf[0:64]
                    a1gv = a1g[:, :].rearrange("p (b c) -> p b c", b=K)
                    for bb in range(K):
                        # start=True resets the whole PSUM bank: only the
                        # first of the 4 sub-bank block matmuls may set it
                        nc.tensor.matmul(psf[0:C, bb * P:(bb + 1) * P],
                                         lhsT=a1gv[:, bb, 0:C],
                                         rhs=identfb[:, :],
                                         start=(bb % 4 == 0), stop=False)
                    for n in range(4):
                        nc.tensor.matmul(psf[0:C, n * 512:(n + 1) * 512],
                                         lhsT=ctab[:, co:co + C],
                                         rhs=sp[:, n * 512:(n + 1) * 512],
                                         start=False, stop=True)
                    for hh in range(2):
                        nc.scalar.activation(
                            hstack[0:C, hh * 1024:(hh + 1) * 1024],
                            psf[0:C, hh * 1024:(hh + 1) * 1024], RELU,
                            bias=b12[0:C, :], scale=1.0)
                    yield
                    # --- stage 2: layer 2 -> psf[64:128]
                    for n in range(4):
                        nc.tensor.matmul(psf[C:P, n * 512:(n + 1) * 512],
                                         lhsT=w2a[:, :],
                                         rhs=hstack[0:C, n * 512:(n + 1) * 512],
                                         start=True, stop=False)
                    for n in range(4):
                        nc.tensor.matmul(psf[C:P, n * 512:(n + 1) * 512],
                                         lhsT=ctab[:, co + C:co + 2 * C],
                                         rhs=sp[:, n * 512:(n + 1) * 512],
                                         start=False, stop=True)
                    for hh in range(2):
                        nc.scalar.activation(
                            hstack[C:P, hh * 1024:(hh + 1) * 1024],
                            psf[C:P, hh * 1024:(hh + 1) * 1024], RELU,
                            bias=b12[C:P, :], scale=1.0)
                    yield
                    # --- stage 3: layer 3 -> psf[0:64]; h1|h2 fold
                    for n in range(4):
                        nc.tensor.matmul(psf[0:C, n * 512:(n + 1) * 512],
                                         lhsT=w3ab[:, :],
                                         rhs=hstack[:, n * 512:(n + 1) * 512],
                                         start=True, stop=False)
                    for n in range(4):
                        nc.tensor.matmul(psf[0:C, n * 512:(n + 1) * 512],
                                         lhsT=ctab[:, co + 2 * C:co + 3 * C],
                                         rhs=sp[:, n * 512:(n + 1) * 512],
                                         start=False, stop=True)
                    _fold(nc, scr, hstack, msbB[:, :], P, 0)
                    for hh in range(2):
                        nc.scalar.activation(
                            h34[0:C, hh * 1024:(hh + 1) * 1024],
                            psf[0:C, hh * 1024:(hh + 1) * 1024], RELU,
                            bias=b43[0:C, :], scale=1.0)
                    yield
                    # --- stage 4: layer 4 -> psf[64:128]
                    for n in range(4):
                        nc.tensor.matmul(psf[C:P, n * 512:(n + 1) * 512],
                                         lhsT=w4a[:, :],
                                         rhs=h34[0:C, n * 512:(n + 1) * 512],
                                         start=True, stop=False)
                    for n in range(4):
                        nc.tensor.matmul(psf[C:P, n * 512:(n + 1) * 512],
                                         lhsT=w4bc[:, :],
                                         rhs=hstack[:, n * 512:(n + 1) * 512],
                                         start=False, stop=True)
                    for hh in range(2):
                        nc.scalar.copy(h34[C:P, hh * 1024:(hh + 1) * 1024],
                                       psf[C:P, hh * 1024:(hh + 1) * 1024])
                    yield
                    # --- stage 5: h3|h4 fold, post-max relu, transpose, out
                    _fold(nc, scr, h34, msbA[:, :], P, E)
                    nc.scalar.activation(msbB[:, :], msbB[:, :], RELU,
                                         bias=b12[:, :], scale=1.0)
                    nc.scalar.activation(msbA[0:C, :], msbA[0:C, :], RELU,
                                         bias=b43[0:C, :], scale=1.0)
                    pso = pps.tile([P, 2 * P], FP, tag="pso", name=f"pso_{u}")
                    nc.tensor.matmul(pso[:, 0:P], lhsT=msbA[:, :],
                                     rhs=identf[:, :], is_transpose=True,
                                     start=True, stop=False)
                    nc.tensor.matmul(pso[:, P:2 * P], lhsT=msbB[:, :],
                                     rhs=identf[:, :], is_transpose=True,
                                     start=False, stop=True)
                    outsb = ring.tile([P, D + 4 * C], FP, tag="outsb")
                    # order: [h4, h3, h2, h1, x]; msbA=[h3;h4], msbB=[h1;h2]
                    nc.scalar.copy(outsb[:, 0:C], pso[:, C:2 * C])
                    nc.scalar.copy(outsb[:, C:2 * C], pso[:, 0:C])
                    nc.scalar.copy(outsb[:, 2 * C:3 * C], pso[:, 3 * C:4 * C])
                    nc.scalar.copy(outsb[:, 3 * C:4 * C], pso[:, 2 * C:3 * C])
                    nc.vector.tensor_tensor(
                        out=outsb[:, 0:C], in0=outsb[:, 0:C],
                        in1=ctab[:, co + 3 * C:co + 4 * C], op=AX.add)
                    yield
                    # --- stage 6 (next iteration): output DMAs, by which
                    #     time the stage-5 compute has long finished, so the
                    #     SP queue never stalls waiting on them.
                    nc.sync.dma_start(outsb[:, 4 * C:4 * C + D],
                                      x_d[u * P:(u + 1) * P, :])
                    nc.sync.dma_start(out_d[u * P:(u + 1) * P, :], outsb[:, :])
                    yield

                def adv(g):
                    if g is not None:
                        next(g, None)

                # KNN mms at t, selection+gather at t-1, FC at t-3.
                a1g_ring = {}
                fc_tail = {}
                for it in range(NT + 4):
                    t, s, u = it, it - 1, it - 3
                    gprev = fc_tail.pop(it - 1, None)
                    adv(gprev)                  # FC(u-1) output DMAs
                    g = fc_stages(u, a1g_ring.pop(u)) if 0 <= u < NT else None
                    if g is not None:
                        fc_tail[it] = g
                    if 0 <= s < NT:
                        a1g_ring[s] = knn_select(s)
                    adv(g)                      # FC(u) stage 1
                    if t < NT:
                        knn_mm_stage(t, 0)
                    adv(g)                      # FC(u) stage 2
                    if t < NT:
                        knn_mm_stage(t, 1)
                    adv(g)                      # FC(u) stage 3
                    if t < NT:
                        knn_mm_stage(t, 2)
                    adv(g)                      # FC(u) stage 4
                    if t < NT:
                        knn_mm_stage(t, 3)
                    adv(g)                      # FC(u) stage 5

    nc.compile()
    return nc


def host_prep(x, pos, W_first, b_first, W_mid1, b_mid1, W_mid2, b_mid2,
              W_last, b_last):
    """Host-side arrangement of per-core inputs (numpy, cheap O(N) work)."""
    import ml_dtypes
    f32 = np.float32
    bf = ml_dtypes.bfloat16
    x = np.asarray(x, f32)
    pos = np.asarray(pos, f32)
    Wf = np.asarray(W_first, f32)
    Wm1 = np.asarray(W_mid1, f32)
    Wm2 = np.asarray(W_mid2, f32)
    Wl = np.asarray(W_last, f32)

    V1 = Wf[D:2 * D] + Wf[2 * D:3 * D]
    U1 = Wf[0:D] - Wf[2 * D:3 * D]
    W2a, W2x = Wm1[0:C], Wm1[C:C + D]
    W3a, W3b, W3c = Wm2[0:C], Wm2[C:2 * C], Wm2[2 * C:2 * C + D]
    W4a, W4b, W4c, W4d = Wl[0:C], Wl[C:2 * C], Wl[2 * C:3 * C], Wl[3 * C:3 * C + D]

    n = x.shape[1]
    b1 = np.asarray(b_first, f32).reshape(C, 1)
    b2 = np.asarray(b_mid1, f32).reshape(C, 1)
    b3v = np.asarray(b_mid2, f32).reshape(C, 1)

    e_idx = np.arange(P * K)
    sp = (np.arange(P)[:, None] == (e_idx // K)[None, :]).astype(bf)

    shared = {
        "wcat": np.concatenate([V1, U1, W2x, W3c, W4d], axis=1).astype(bf),
        "w2a": W2a.astype(bf),
        # hstack rows: [h1 (0:64); h2 (64:128)]
        "w3ab": np.concatenate([W3b, W3a], axis=0).astype(bf),
        "w4a": W4a.astype(bf),
        "w4bc": np.concatenate([W4c, W4b], axis=0).astype(bf),
        "b12": np.concatenate([b1, b2], axis=0).copy(),
        "b43": np.concatenate([b3v, np.zeros_like(b3v)], axis=0).copy(),
        "b4rb": np.broadcast_to(np.asarray(b_last, f32).reshape(1, C),
                                (P, C)).astype(bf),
        "sp": np.ascontiguousarray(sp),
        "identfb": np.eye(P, dtype=f32).astype(bf),
        "identf": np.eye(P, dtype=f32),
        "ibig": (np.eye(P, dtype=f32) * 1.0e38),
        "iota": np.broadcast_to(np.arange(n, dtype=np.uint32), (P, n)).copy(),
    }

    in_maps = []
    for bi in range(x.shape[0]):
        pb = pos[bi]                                  # (N, 3)
        sq = (pb * pb).sum(axis=-1, dtype=f32)        # (N,)
        ph = pb.astype(bf)
        pl = (pb - ph.astype(f32)).astype(bf)
        sqh = sq.astype(bf)
        sql = (sq - sqh.astype(f32)).astype(bf)
        ones = np.ones(n, f32)

        ka = np.zeros((16, n), f32)
        kb = np.zeros((16, n), f32)
        ph32, pl32 = ph.astype(f32), pl.astype(f32)
        ka[0:3] = 2.0 * ph32.T
        kb[0:3] = ph32.T
        ka[3:6] = 2.0 * ph32.T
        kb[3:6] = pl32.T
        ka[6:9] = 2.0 * pl32.T
        kb[6:9] = ph32.T
        ka[9] = sqh.astype(f32)
        kb[9] = -ones
        ka[10] = sql.astype(f32)
        kb[10] = -ones
        ka[11] = -ones
        kb[11] = sqh.astype(f32)
        ka[12] = -ones
        kb[12] = sql.astype(f32)

        m = dict(shared)
        m["x"] = np.ascontiguousarray(x[bi])
        m["xt"] = np.ascontiguousarray(x[bi].T).astype(bf)
        m["ka"] = ka.astype(bf)
        m["kb"] = kb.astype(bf)
        in_maps.append(m)
    return in_maps


_NC_CACHE = {}
LAST_RESULT = None


def kernel(**inputs):
    import os

    from concourse.bass_utils import run_bass_kernel_spmd

    global LAST_RESULT
    in_maps = host_prep(**inputs)
    n = inputs["x"].shape[1]
    if n not in _NC_CACHE:
        _NC_CACHE[n] = build_kernel(n)
    nc = _NC_CACHE[n]
    trace = bool(os.environ.get("KERNEL_TRACE"))
    res = run_bass_kernel_spmd(nc, in_maps, core_ids=list(range(len(in_maps))),
                               trace=trace)
    LAST_RESULT = res
    out = np.stack([r["out"] for r in res.results], axis=0)
    return out


# revision 14
# speedup vs baseline: 1.3634x; 1.0772x over previous
# DenseEdgeConv (B=8, N=4096, D=128, K=16, C=64) Trainium2 Bass kernel, v3.
#
# Data-parallel over B (one point cloud per core). Per core:
#
#   KNN:  ds = -d2 computed on the PE as a 13-row bf16 hi/lo-split matmul
#         (products hi*hi + hi*lo + lo*hi; sq_i/sq_j split hi+lo; error
#         ~2^-17, far below the rank-16/17 distance gap). The DVE then
#         "twiddles" ds: low 12 mantissa bits := column index j (one
#         scalar_tensor_tensor pass, PSUM->SBUF). Ordering keeps 11
#         mantissa bits -- ample for neighbor selection. Top-16 per row
#         via 8 per-512-chunk max8 calls + top-16 of the 64 candidates;
#         neighbor indices fall out of the low bits of the winners (no
#         full-row max_index passes).
#   Gather: indices go to the [16,128]-replicated layout with one XBAR
#         dma transpose, then 4 batched transpose dma_gathers pull the
#         2048 neighbor rows of the bf16 a1-table directly into the
#         transposed [64ch, 2048edge] layout.
#   FC:   algebraic restructure (per-point tables a1/c1..c4):
#           h1 = relu(a1[j] + c1[i] + b1)
#           h2 = relu(W2a^T h1 + c2[i] + b2)
#           h3 = relu(W3a^T h2 + W3b^T h1 + c3[i] + b3)
#           h4 = W4a^T h3 + W4b^T h2 + W4c^T h1   (+ c4[i] + b4 post-max)
#         All matmuls bf16 (1 cycle/row); c-terms broadcast over k with
#         S' = I128 (x) ones(16) selection matmuls. Edge order e = i*16+k
#         (k innermost) so max over k is a contiguous fold, done as bf16
#         tensor-tensor max folds. relu/bias commute with the max
#         (monotone, per-point constant). h1/h2 share one [128, E] tile
#         and h3/h4 another, so each fold covers two layers.
#   out[i] = [h4max + c4 + b4, h3max, h2max, h1max, x[i]]
#
# The per-tile work is software-pipelined: FC lags KNN by 2 tiles and FC
# stages are emitted interleaved with the KNN matmul quarters so no
# engine queue head-blocks.

import numpy as np

import concourse.bacc as bacc
import concourse.bass as bass
import concourse.mybir as mybir
import concourse.tile as tile

FP = mybir.dt.float32
BF = mybir.dt.bfloat16
U32 = mybir.dt.uint32
U16 = mybir.dt.uint16
I16 = mybir.dt.int16

B, N_FULL, D, K, C = 8, 4096, 128, 16, 64
P = 128
E = P * K
AX = mybir.AluOpType
RELU = mybir.ActivationFunctionType.Relu


def _fold(nc, scr, h, out_ap, rows, s0):
    """max over k=16 (innermost, contiguous) of h [rows, (i k)] -> out [rows, 128]."""
    v = h[0:rows, :].rearrange("c (i k) -> c i k", k=16)
    f1 = scr[0:rows, s0:s0 + E // 2].rearrange("c (i k) -> c i k", k=8)
    nc.vector.tensor_tensor(out=f1, in0=v[:, :, 0:8], in1=v[:, :, 8:16], op=AX.max)
    f2 = scr[0:rows, s0 + E // 2:s0 + 3 * E // 4].rearrange("c (i k) -> c i k", k=4)
    nc.vector.tensor_tensor(out=f2, in0=f1[:, :, 0:4], in1=f1[:, :, 4:8], op=AX.max)
    f3 = scr[0:rows, s0 + 3 * E // 4:s0 + 7 * E // 8].rearrange("c (i k) -> c i k", k=2)
    nc.vector.tensor_tensor(out=f3, in0=f2[:, :, 0:2], in1=f2[:, :, 2:4], op=AX.max)
    nc.vector.tensor_tensor(out=out_ap, in0=f3[:, :, 0], in1=f3[:, :, 1], op=AX.max)


def build_kernel(N=N_FULL):
    NT = N // P          # 32 point tiles
    NCH = 8              # knn max8 chunks per row
    CHW = N // NCH       # 512

    nc = bacc.Bacc("TRN2", target_bir_lowering=False, debug=False)

    x_d = nc.dram_tensor("x", [N, D], FP, kind="ExternalInput").ap()
    xt_d = nc.dram_tensor("xt", [D, N], BF, kind="ExternalInput").ap()
    ka_d = nc.dram_tensor("ka", [16, N], BF, kind="ExternalInput").ap()
    kb_d = nc.dram_tensor("kb", [16, N], BF, kind="ExternalInput").ap()
    wcat_d = nc.dram_tensor("wcat", [D, 320], BF, kind="ExternalInput").ap()
    w2a_d = nc.dram_tensor("w2a", [C, C], BF, kind="ExternalInput").ap()
    w3ab_d = nc.dram_tensor("w3ab", [2 * C, C], BF, kind="ExternalInput").ap()
    w4a_d = nc.dram_tensor("w4a", [C, C], BF, kind="ExternalInput").ap()
    w4bc_d = nc.dram_tensor("w4bc", [2 * C, C], BF, kind="ExternalInput").ap()
    b12_d = nc.dram_tensor("b12", [P, 1], FP, kind="ExternalInput").ap()
    b43_d = nc.dram_tensor("b43", [P, 1], FP, kind="ExternalInput").ap()
    b4rb_d = nc.dram_tensor("b4rb", [P, C], BF, kind="ExternalInput").ap()
    sp_d = nc.dram_tensor("sp", [P, E], BF, kind="ExternalInput").ap()
    identfb_d = nc.dram_tensor("identfb", [P, P], BF, kind="ExternalInput").ap()
    identf_d = nc.dram_tensor("identf", [P, P], FP, kind="ExternalInput").ap()
    ibig_d = nc.dram_tensor("ibig", [P, P], FP, kind="ExternalInput").ap()
    iota_d = nc.dram_tensor("iota", [P, N], U32, kind="ExternalInput").ap()

    out_d = nc.dram_tensor("out", [N, D + 4 * C], FP, kind="ExternalOutput").ap()

    with tile.TileContext(nc) as tc:
        with (
            tc.tile_pool(name="const", bufs=1) as cpool,
            tc.tile_pool(name="persist", bufs=1) as ppool,
            tc.tile_pool(name="dram", bufs=1, space="DRAM") as dpool,
        ):
            def cin(name, shape, dt, src):
                tl = cpool.tile(shape, dt, name=name)
                nc.sync.dma_start(tl[:, :], src)
                return tl

            ka = cin("ka", [16, N], BF, ka_d)
            kb = cin("kb", [16, N], BF, kb_d)
            wcat = cin("wcat", [D, 320], BF, wcat_d)
            w2a = cin("w2a", [C, C], BF, w2a_d)
            w3ab = cin("w3ab", [2 * C, C], BF, w3ab_d)
            w4a = cin("w4a", [C, C], BF, w4a_d)
            w4bc = cin("w4bc", [2 * C, C], BF, w4bc_d)
            b12 = cin("b12", [P, 1], FP, b12_d)
            b43 = cin("b43", [P, 1], FP, b43_d)
            b4rb = cin("b4rb", [P, C], BF, b4rb_d)
            sp = cin("sp", [P, E], BF, sp_d)
            identfb = cin("identfb", [P, P], BF, identfb_d)
            identf = cin("identf", [P, P], FP, identf_d)
            ibig = cin("ibig", [P, P], FP, ibig_d)
            iota = cin("iota", [P, N], U32, iota_d)
            msk = cpool.tile([P, 1], U32)
            nc.gpsimd.memset(msk[:, :], int(0xFFFFF000))
            msk12 = cpool.tile([P, 1], U32)
            nc.gpsimd.memset(msk12[:, :], int(0xFFF))

            ctab = ppool.tile([P, NT * 256], BF)   # [c1|c2|c3|c4] per tile
            a1bf = dpool.tile([N, P], BF)          # a1 rows padded to 256B

            # ============ Phase A: per-point tables ============
            with (
                tc.tile_pool(name="axt", bufs=1) as axt,
                tc.tile_pool(name="apsum", bufs=2, space="PSUM") as aps,
                tc.tile_pool(name="asb", bufs=2) as asb,
            ):
                xt = axt.tile([D, N], BF)
                nc.sync.dma_start(xt[:, :], xt_d)
                for t in range(NT):
                    cps = aps.tile([P, 320], FP, tag="cps")
                    nc.tensor.matmul(cps[:, :], lhsT=xt[:, t * P:(t + 1) * P],
                                     rhs=wcat[:, :], start=True, stop=True)
                    a1s = asb.tile([P, P], BF, tag="a1s")
                    nc.gpsimd.memset(a1s[:, C:P], 0.0)
                    nc.scalar.copy(a1s[:, 0:C], cps[:, 0:C])
                    nc.sync.dma_start(a1bf[t * P:(t + 1) * P, :], a1s[:, :])
                    nc.scalar.copy(ctab[:, t * 256:(t + 1) * 256], cps[:, C:320])
                    nc.vector.tensor_tensor(
                        out=ctab[:, t * 256 + 192:t * 256 + 256],
                        in0=ctab[:, t * 256 + 192:t * 256 + 256],
                        in1=b4rb[:, :], op=AX.add)

            # ============ Phase B: fused KNN + FC, software-pipelined ======
            with (
                tc.tile_pool(name="kps", bufs=1, space="PSUM") as kps,
                tc.tile_pool(name="pps", bufs=2, space="PSUM") as pps,
                tc.tile_pool(name="fps", bufs=1, space="PSUM") as fps,
                tc.tile_pool(name="knnsb", bufs=1) as ksb,
                tc.tile_pool(name="ring", bufs=2) as ring,
            ):
                t12 = ksb.tile([P, 16], FP)
                ix = ksb.tile([P, 16], U32)
                ds_ring = {}
                cand_ring = {}

                def knn_mm_stage(t, q):
                    """One PSUM quarter of distances, twiddle, chunk maxes."""
                    if q == 0:
                        ds_ring[t] = ring.tile([P, N], FP, tag="ds",
                                               name=f"ds_{t}")
                        cand_ring[t] = ring.tile([P, NCH * 8], FP, tag="cand",
                                                 name=f"cand_{t}")
                    ds = ds_ring[t]
                    cand = cand_ring[t]
                    dq = kps.tile([P, 1024], FP, tag="knn", name=f"dq_{t}_{q}")
                    for h in range(2):
                        c0 = h * 512
                        nc.tensor.matmul(dq[:, c0:c0 + 512],
                                         lhsT=ka[:, t * P:(t + 1) * P],
                                         rhs=kb[:, q * 1024 + c0:q * 1024 + c0 + 512],
                                         start=True, stop=True)
                    nc.vector.scalar_tensor_tensor(
                        out=ds[:, q * 1024:(q + 1) * 1024].bitcast(U32),
                        in0=dq[:, :].bitcast(U32), scalar=msk[:, :],
                        in1=iota[:, q * 1024:(q + 1) * 1024],
                        op0=AX.bitwise_and, op1=AX.bitwise_or)
                    if q == t // 8:
                        # self-distance kill lives in this quarter
                        nc.vector.tensor_tensor(out=ds[:, t * P:(t + 1) * P],
                                                in0=ds[:, t * P:(t + 1) * P],
                                                in1=ibig[:, :], op=AX.subtract)
                    for ch in (2 * q, 2 * q + 1):
                        nc.vector.max(cand[:, ch * 8:(ch + 1) * 8],
                                      ds[:, ch * CHW:(ch + 1) * CHW])

                def knn_select(s):
                    """Top-16 of the chunk candidates, index extract, gather."""
                    ds_ring.pop(s)
                    cand = cand_ring.pop(s)
                    nc.vector.max(t12[:, 0:8], cand[:, :])
                    nc.vector.match_replace(out=cand[:, :],
                                            in_to_replace=t12[:, 0:8],
                                            in_values=cand[:, :],
                                            imm_value=-1.0e30)
                    nc.vector.max(t12[:, 8:16], cand[:, :])
                    nc.vector.tensor_scalar(out=ix[:, :],
                                            in0=t12[:, :].bitcast(U32),
                                            scalar1=msk12[:, :], scalar2=None,
                                            op0=AX.bitwise_and)
                    idxJ = ring.tile([P, P], I16, tag="idxJ")
                    nc.vector.tensor_copy(
                        idxJ[:, 0:16].bitcast(U16),
                        ix[:, :].bitcast(U16)
                        .rearrange("p (k two) -> p k two", two=2)[:, :, 0])
                    nc.vector.tensor_copy(idxJ[:, 16:32], idxJ[:, 0:16])
                    nc.vector.tensor_copy(idxJ[:, 32:64], idxJ[:, 0:32])
                    nc.vector.tensor_copy(idxJ[:, 64:128], idxJ[:, 0:64])
                    idxT = ring.tile([P, P], I16, tag="idxT")
                    nc.sync.dma_start_transpose(idxT[:, :], idxJ[:, :])
                    a1g = ring.tile([P, K * P], BF, tag="a1g", bufs=3)
                    a1gv = a1g[:, :].rearrange("p (b c) -> p b c", b=K)
                    for g in range(2):
                        nc.gpsimd.dma_gather(
                            out_ap=a1gv[:, g * 8:(g + 1) * 8, :],
                            in_ap=a1bf[:, :],
                            idxs_ap=idxT[:, g * 64:(g + 1) * 64],
                            num_idxs=1024, num_idxs_reg=1024, elem_size=P,
                            transpose=False)
                    return a1g

                def fc_stages(u, a1g):
                    """Generator: one FC tile in 5 stages (yield between them)."""
                    co = u * 256
                    hstack = ring.tile([P, E], BF, tag="hstack")
                    h34 = ring.tile([P, E], BF, tag="h34")
                    scr = ring.tile([P, 2 * E], BF, tag="scr")
                    msbA = ring.tile([P, P], FP, tag="msbA")
                    msbB = ring.tile([P, P], FP, tag="msbB")
                    psf = fps.tile([P, E], FP, tag="fc", name=f"psf_{u}")

                    # --- stage 1: joint c1|c2 broadcast (start) then the
                    #     a1g transposes accumulate layer 1 in psf[0:64];
                    #     psf[64:128] holds c2 awaiting layer 2.
                    a1gv = a1g[:, :].rearrange("p (b c) -> p b c", b=K)
                    for n in range(4):
                        nc.tensor.matmul(psf[:, n * 512:(n + 1) * 512],
                                         lhsT=ctab[:, co:co + 2 * C],
                                         rhs=sp[:, n * 512:(n + 1) * 512],
                                         start=True, stop=False)
                    for bb in range(K):
                        nc.tensor.matmul(psf[0:C, bb * P:(bb + 1) * P],
                                         lhsT=a1gv[:, bb, 0:C],
                                         rhs=identfb[:, :],
                                         start=False, stop=(bb % 4 == 3))
                    for hh in range(2):
                        nc.scalar.activation(
                            hstack[0:C, hh * 1024:(hh + 1) * 1024],
                            psf[0:C, hh * 1024:(hh + 1) * 1024], RELU,
                            bias=b12[0:C, :], scale=1.0)
                    yield
                    # --- stage 2: layer 2 accumulates onto c2 in psf[64:128]
                    for n in range(4):
                        nc.tensor.matmul(psf[C:P, n * 512:(n + 1) * 512],
                                         lhsT=w2a[:, :],
                                         rhs=hstack[0:C, n * 512:(n + 1) * 512],
                                         start=False, stop=True)
                    for hh in range(2):
                        nc.scalar.activation(
                            hstack[C:P, hh * 1024:(hh + 1) * 1024],
                            psf[C:P, hh * 1024:(hh + 1) * 1024], RELU,
                            bias=b12[C:P, :], scale=1.0)
                    yield
                    # --- stage 3: layer 3 -> psf[0:64]; h1|h2 fold
                    for n in range(4):
                        nc.tensor.matmul(psf[0:C, n * 512:(n + 1) * 512],
                                         lhsT=w3ab[:, :],
                                         rhs=hstack[:, n * 512:(n + 1) * 512],
                                         start=True, stop=False)
                    for n in range(4):
                        nc.tensor.matmul(psf[0:C, n * 512:(n + 1) * 512],
                                         lhsT=ctab[:, co + 2 * C:co + 3 * C],
                                         rhs=sp[:, n * 512:(n + 1) * 512],
                                         start=False, stop=True)
                    _fold(nc, scr, hstack, msbB[:, :], P, 0)
                    for hh in range(2):
                        nc.scalar.activation(
                            h34[0:C, hh * 1024:(hh + 1) * 1024],
                            psf[0:C, hh * 1024:(hh + 1) * 1024], RELU,
                            bias=b43[0:C, :], scale=1.0)
                    yield
                    # --- stage 4: layer 4 -> psf[64:128]
                    for n in range(4):
                        nc.tensor.matmul(psf[C:P, n * 512:(n + 1) * 512],
                                         lhsT=w4a[:, :],
                                         rhs=h34[0:C, n * 512:(n + 1) * 512],
                                         start=True, stop=False)
                    for n in range(4):
                        nc.tensor.matmul(psf[C:P, n * 512:(n + 1) * 512],
                                         lhsT=w4bc[:, :],
                                         rhs=hstack[:, n * 512:(n + 1) * 512],
                                         start=False, stop=True)
                    for hh in range(2):
                        nc.scalar.copy(h34[C:P, hh * 1024:(hh + 1) * 1024],
                                       psf[C:P, hh * 1024:(hh + 1) * 1024])
                    yield
                    # --- stage 5: h3|h4 fold, post-max relu, transpose, out
                    _fold(nc, scr, h34, msbA[:, :], P, E)
                    nc.scalar.activation(msbB[:, :], msbB[:, :], RELU,
                                         bias=b12[:, :], scale=1.0)
                    nc.scalar.activation(msbA[0:C, :], msbA[0:C, :], RELU,
                                         bias=b43[0:C, :], scale=1.0)
                    pso = pps.tile([P, 2 * P], FP, tag="pso", name=f"pso_{u}")
                    nc.tensor.matmul(pso[:, 0:P], lhsT=msbA[:, :],
                                     rhs=identf[:, :], is_transpose=True,
                                     start=True, stop=False)
                    nc.tensor.matmul(pso[:, P:2 * P], lhsT=msbB[:, :],
                                     rhs=identf[:, :], is_transpose=True,
                                     start=False, stop=True)
                    outsb = ring.tile([P, D + 4 * C], FP, tag="outsb")
                    # order: [h4, h3, h2, h1, x]; msbA=[h3;h4], msbB=[h1;h2]
                    nc.scalar.copy(outsb[:, 0:C], pso[:, C:2 * C])
                    nc.scalar.copy(outsb[:, C:2 * C], pso[:, 0:C])
                    nc.scalar.copy(outsb[:, 2 * C:3 * C], pso[:, 3 * C:4 * C])
                    nc.scalar.copy(outsb[:, 3 * C:4 * C], pso[:, 2 * C:3 * C])
                    nc.vector.tensor_tensor(
                        out=outsb[:, 0:C], in0=outsb[:, 0:C],
                        in1=ctab[:, co + 3 * C:co + 4 * C], op=AX.add)
                    yield
                    # --- stage 6 (next iteration): output DMAs, by which
                    #     time the stage-5 compute has long finished, so the
                    #     SP queue never stalls waiting on them.
                    nc.sync.dma_start(outsb[:, 4 * C:4 * C + D],
                                      x_d[u * P:(u + 1) * P, :])
                    nc.sync.dma_start(out_d[u * P:(u + 1) * P, :], outsb[:, :])
                    yield

                def adv(g):
                    if g is not None:
                        next(g, None)

                # KNN mms at t, selection+gather at t-1, FC at t-3.
                a1g_ring = {}
                fc_tail = {}
                for it in range(NT + 4):
                    t, s, u = it, it - 1, it - 3
                    gprev = fc_tail.pop(it - 1, None)
                    adv(gprev)                  # FC(u-1) output DMAs
                    g = fc_stages(u, a1g_ring.pop(u)) if 0 <= u < NT else None
                    if g is not None:
                        fc_tail[it] = g
                    if 0 <= s < NT:
                        a1g_ring[s] = knn_select(s)
                    adv(g)                      # FC(u) stage 1
                    if t < NT:
                        knn_mm_stage(t, 0)
                    adv(g)                      # FC(u) stage 2
                    if t < NT:
                        knn_mm_stage(t, 1)
                    adv(g)                      # FC(u) stage 3
                    if t < NT:
                        knn_mm_stage(t, 2)
                    adv(g)                      # FC(u) stage 4
                    if t < NT:
                        knn_mm_stage(t, 3)
                    adv(g)                      # FC(u) stage 5

    nc.compile()
    return nc


def host_prep(x, pos, W_first, b_first, W_mid1, b_mid1, W_mid2, b_mid2,
              W_last, b_last):
    """Host-side arrangement of per-core inputs (numpy, cheap O(N) work)."""
    import ml_dtypes
    f32 = np.float32
    bf = ml_dtypes.bfloat16
    x = np.asarray(x, f32)
    pos = np.asarray(pos, f32)
    Wf = np.asarray(W_first, f32)
    Wm1 = np.asarray(W_mid1, f32)
    Wm2 = np.asarray(W_mid2, f32)
    Wl = np.asarray(W_last, f32)

    V1 = Wf[D:2 * D] + Wf[2 * D:3 * D]
    U1 = Wf[0:D] - Wf[2 * D:3 * D]
    W2a, W2x = Wm1[0:C], Wm1[C:C + D]
    W3a, W3b, W3c = Wm2[0:C], Wm2[C:2 * C], Wm2[2 * C:2 * C + D]
    W4a, W4b, W4c, W4d = Wl[0:C], Wl[C:2 * C], Wl[2 * C:3 * C], Wl[3 * C:3 * C + D]

    n = x.shape[1]
    b1 = np.asarray(b_first, f32).reshape(C, 1)
    b2 = np.asarray(b_mid1, f32).reshape(C, 1)
    b3v = np.asarray(b_mid2, f32).reshape(C, 1)

    e_idx = np.arange(P * K)
    sp = (np.arange(P)[:, None] == (e_idx // K)[None, :]).astype(bf)

    shared = {
        "wcat": np.concatenate([V1, U1, W2x, W3c, W4d], axis=1).astype(bf),
        "w2a": W2a.astype(bf),
        # hstack rows: [h1 (0:64); h2 (64:128)]
        "w3ab": np.concatenate([W3b, W3a], axis=0).astype(bf),
        "w4a": W4a.astype(bf),
        "w4bc": np.concatenate([W4c, W4b], axis=0).astype(bf),
        "b12": np.concatenate([b1, b2], axis=0).copy(),
        "b43": np.concatenate([b3v, np.zeros_like(b3v)], axis=0).copy(),
        "b4rb": np.broadcast_to(np.asarray(b_last, f32).reshape(1, C),
                                (P, C)).astype(bf),
        "sp": np.ascontiguousarray(sp),
        "identfb": np.eye(P, dtype=f32).astype(bf),
        "identf": np.eye(P, dtype=f32),
        "ibig": (np.eye(P, dtype=f32) * 1.0e38),
        "iota": np.broadcast_to(np.arange(n, dtype=np.uint32), (P, n)).copy(),
    }

    in_maps = []
    for bi in range(x.shape[0]):
        pb = pos[bi]                                  # (N, 3)
        sq = (pb * pb).sum(axis=-1, dtype=f32)        # (N,)
        ph = pb.astype(bf)
        pl = (pb - ph.astype(f32)).astype(bf)
        sqh = sq.astype(bf)
        sql = (sq - sqh.astype(f32)).astype(bf)
        ones = np.ones(n, f32)

        ka = np.zeros((16, n), f32)
        kb = np.zeros((16, n), f32)
        ph32, pl32 = ph.astype(f32), pl.astype(f32)
        ka[0:3] = 2.0 * ph32.T
        kb[0:3] = ph32.T
        ka[3:6] = 2.0 * ph32.T
        kb[3:6] = pl32.T
        ka[6:9] = 2.0 * pl32.T
        kb[6:9] = ph32.T
        ka[9] = sqh.astype(f32)
        kb[9] = -ones
        ka[10] = sql.astype(f32)
        kb[10] = -ones
        ka[11] = -ones
        kb[11] = sqh.astype(f32)
        ka[12] = -ones
        kb[12] = sql.astype(f32)

        m = dict(shared)
        m["x"] = np.ascontiguousarray(x[bi])
        m["xt"] = np.ascontiguousarray(x[bi].T).astype(bf)
        m["ka"] = ka.astype(bf)
        m["kb"] = kb.astype(bf)
        in_maps.append(m)
    return in_maps


_NC_CACHE = {}
LAST_RESULT = None


def kernel(**inputs):
    import os

    from concourse.bass_utils import run_bass_kernel_spmd

    global LAST_RESULT
    in_maps = host_prep(**inputs)
    n = inputs["x"].shape[1]
    if n not in _NC_CACHE:
        _NC_CACHE[n] = build_kernel(n)
    nc = _NC_CACHE[n]
    trace = bool(os.environ.get("KERNEL_TRACE"))
    res = run_bass_kernel_spmd(nc, in_maps, core_ids=list(range(len(in_maps))),
                               trace=trace)
    LAST_RESULT = res
    out = np.stack([r["out"] for r in res.results], axis=0)
    return out


# revision 15
# speedup vs baseline: 1.4142x; 1.0372x over previous
# DenseEdgeConv (B=8, N=4096, D=128, K=16, C=64) Trainium2 Bass kernel, v3.
#
# Data-parallel over B (one point cloud per core). Per core:
#
#   KNN:  ds = -d2 computed on the PE as a 13-row bf16 hi/lo-split matmul
#         (products hi*hi + hi*lo + lo*hi; sq_i/sq_j split hi+lo; error
#         ~2^-17, far below the rank-16/17 distance gap). The DVE then
#         "twiddles" ds: low 12 mantissa bits := column index j (one
#         scalar_tensor_tensor pass, PSUM->SBUF). Ordering keeps 11
#         mantissa bits -- ample for neighbor selection. Top-16 per row
#         via 8 per-512-chunk max8 calls + top-16 of the 64 candidates;
#         neighbor indices fall out of the low bits of the winners (no
#         full-row max_index passes).
#   Gather: indices go to the [16,128]-replicated layout with one XBAR
#         dma transpose, then 4 batched transpose dma_gathers pull the
#         2048 neighbor rows of the bf16 a1-table directly into the
#         transposed [64ch, 2048edge] layout.
#   FC:   algebraic restructure (per-point tables a1/c1..c4):
#           h1 = relu(a1[j] + c1[i] + b1)
#           h2 = relu(W2a^T h1 + c2[i] + b2)
#           h3 = relu(W3a^T h2 + W3b^T h1 + c3[i] + b3)
#           h4 = W4a^T h3 + W4b^T h2 + W4c^T h1   (+ c4[i] + b4 post-max)
#         All matmuls bf16 (1 cycle/row); c-terms broadcast over k with
#         S' = I128 (x) ones(16) selection matmuls. Edge order e = i*16+k
#         (k innermost) so max over k is a contiguous fold, done as bf16
#         tensor-tensor max folds. relu/bias commute with the max
#         (monotone, per-point constant). h1/h2 share one [128, E] tile
#         and h3/h4 another, so each fold covers two layers.
#   out[i] = [h4max + c4 + b4, h3max, h2max, h1max, x[i]]
#
# The per-tile work is software-pipelined: FC lags KNN by 2 tiles and FC
# stages are emitted interleaved with the KNN matmul quarters so no
# engine queue head-blocks.

import numpy as np

import concourse.bacc as bacc
import concourse.bass as bass
import concourse.mybir as mybir
import concourse.tile as tile

FP = mybir.dt.float32
BF = mybir.dt.bfloat16
U32 = mybir.dt.uint32
U16 = mybir.dt.uint16
I16 = mybir.dt.int16

B, N_FULL, D, K, C = 8, 4096, 128, 16, 64
P = 128
E = P * K
AX = mybir.AluOpType
RELU = mybir.ActivationFunctionType.Relu


def _fold(nc, scr, h, out_ap, rows, s0):
    """max over k=16 (innermost, contiguous) of h [rows, (i k)] -> out [rows, 128]."""
    v = h[0:rows, :].rearrange("c (i k) -> c i k", k=16)
    f1 = scr[0:rows, s0:s0 + E // 2].rearrange("c (i k) -> c i k", k=8)
    nc.vector.tensor_tensor(out=f1, in0=v[:, :, 0:8], in1=v[:, :, 8:16], op=AX.max)
    f2 = scr[0:rows, s0 + E // 2:s0 + 3 * E // 4].rearrange("c (i k) -> c i k", k=4)
    nc.vector.tensor_tensor(out=f2, in0=f1[:, :, 0:4], in1=f1[:, :, 4:8], op=AX.max)
    f3 = scr[0:rows, s0 + 3 * E // 4:s0 + 7 * E // 8].rearrange("c (i k) -> c i k", k=2)
    nc.vector.tensor_tensor(out=f3, in0=f2[:, :, 0:2], in1=f2[:, :, 2:4], op=AX.max)
    nc.vector.tensor_tensor(out=out_ap, in0=f3[:, :, 0], in1=f3[:, :, 1], op=AX.max)


def build_kernel(N=N_FULL):
    NT = N // P          # 32 point tiles
    NCH = 8              # knn max8 chunks per row
    CHW = N // NCH       # 512

    nc = bacc.Bacc("TRN2", target_bir_lowering=False, debug=False)

    x_d = nc.dram_tensor("x", [N, D], FP, kind="ExternalInput").ap()
    xt_d = nc.dram_tensor("xt", [D, N], BF, kind="ExternalInput").ap()
    ka_d = nc.dram_tensor("ka", [16, N], BF, kind="ExternalInput").ap()
    kb_d = nc.dram_tensor("kb", [16, N], BF, kind="ExternalInput").ap()
    wcat_d = nc.dram_tensor("wcat", [D, 320], BF, kind="ExternalInput").ap()
    w2a_d = nc.dram_tensor("w2a", [C, C], BF, kind="ExternalInput").ap()
    w3ab_d = nc.dram_tensor("w3ab", [2 * C, C], BF, kind="ExternalInput").ap()
    w4a_d = nc.dram_tensor("w4a", [C, C], BF, kind="ExternalInput").ap()
    w4bc_d = nc.dram_tensor("w4bc", [2 * C, C], BF, kind="ExternalInput").ap()
    b12_d = nc.dram_tensor("b12", [P, 1], FP, kind="ExternalInput").ap()
    b43_d = nc.dram_tensor("b43", [P, 1], FP, kind="ExternalInput").ap()
    b4rb_d = nc.dram_tensor("b4rb", [P, C], BF, kind="ExternalInput").ap()
    sp_d = nc.dram_tensor("sp", [P, E], BF, kind="ExternalInput").ap()
    identfb_d = nc.dram_tensor("identfb", [P, P], BF, kind="ExternalInput").ap()
    identf_d = nc.dram_tensor("identf", [P, P], FP, kind="ExternalInput").ap()
    ibig_d = nc.dram_tensor("ibig", [P, P], FP, kind="ExternalInput").ap()
    iota_d = nc.dram_tensor("iota", [P, N], U32, kind="ExternalInput").ap()

    out_d = nc.dram_tensor("out", [N, D + 4 * C], FP, kind="ExternalOutput").ap()

    with tile.TileContext(nc) as tc:
        with (
            tc.tile_pool(name="const", bufs=1) as cpool,
            tc.tile_pool(name="persist", bufs=1) as ppool,
            tc.tile_pool(name="dram", bufs=1, space="DRAM") as dpool,
        ):
            def cin(name, shape, dt, src):
                tl = cpool.tile(shape, dt, name=name)
                nc.sync.dma_start(tl[:, :], src)
                return tl

            ka = cin("ka", [16, N], BF, ka_d)
            kb = cin("kb", [16, N], BF, kb_d)
            wcat = cin("wcat", [D, 320], BF, wcat_d)
            w2a = cin("w2a", [C, C], BF, w2a_d)
            w3ab = cin("w3ab", [2 * C, C], BF, w3ab_d)
            w4a = cin("w4a", [C, C], BF, w4a_d)
            w4bc = cin("w4bc", [2 * C, C], BF, w4bc_d)
            b12 = cin("b12", [P, 1], FP, b12_d)
            b43 = cin("b43", [P, 1], FP, b43_d)
            b4rb = cin("b4rb", [P, C], BF, b4rb_d)
            sp = cin("sp", [P, E], BF, sp_d)
            identfb = cin("identfb", [P, P], BF, identfb_d)
            identf = cin("identf", [P, P], FP, identf_d)
            ibig = cin("ibig", [P, P], FP, ibig_d)
            iota = cin("iota", [P, N], U32, iota_d)
            msk = cpool.tile([P, 1], U32)
            nc.gpsimd.memset(msk[:, :], int(0xFFFFF000))
            msk12 = cpool.tile([P, 1], U32)
            nc.gpsimd.memset(msk12[:, :], int(0xFFF))

            ctab = ppool.tile([P, NT * 256], BF)   # [c1|c2|c3|c4] per tile
            a1bf = dpool.tile([N, P], BF)          # a1 rows padded to 256B

            # ============ Phase A: per-point tables ============
            with (
                tc.tile_pool(name="axt", bufs=1) as axt,
                tc.tile_pool(name="apsum", bufs=2, space="PSUM") as aps,
                tc.tile_pool(name="asb", bufs=2) as asb,
            ):
                xt = axt.tile([D, N], BF)
                nc.sync.dma_start(xt[:, :], xt_d)
                for t in range(NT):
                    cps = aps.tile([P, 320], FP, tag="cps")
                    nc.tensor.matmul(cps[:, :], lhsT=xt[:, t * P:(t + 1) * P],
                                     rhs=wcat[:, :], start=True, stop=True)
                    a1s = asb.tile([P, P], BF, tag="a1s")
                    nc.gpsimd.memset(a1s[:, C:P], 0.0)
                    nc.scalar.copy(a1s[:, 0:C], cps[:, 0:C])
                    nc.sync.dma_start(a1bf[t * P:(t + 1) * P, :], a1s[:, :])
                    nc.scalar.copy(ctab[:, t * 256:(t + 1) * 256], cps[:, C:320])
                    nc.vector.tensor_tensor(
                        out=ctab[:, t * 256 + 192:t * 256 + 256],
                        in0=ctab[:, t * 256 + 192:t * 256 + 256],
                        in1=b4rb[:, :], op=AX.add)

            # ============ Phase B: fused KNN + FC, software-pipelined ======
            with (
                tc.tile_pool(name="kps", bufs=1, space="PSUM") as kps,
                tc.tile_pool(name="pps", bufs=2, space="PSUM") as pps,
                tc.tile_pool(name="fps", bufs=1, space="PSUM") as fps,
                tc.tile_pool(name="knnsb", bufs=1) as ksb,
                tc.tile_pool(name="ring", bufs=2) as ring,
            ):
                t12 = ksb.tile([P, 16], FP)
                ix = ksb.tile([P, 16], U32)
                ds_ring = {}
                cand_ring = {}

                def knn_mm_stage(t, q):
                    """One PSUM quarter of distances, twiddle, chunk maxes."""
                    if q == 0:
                        ds_ring[t] = ring.tile([P, N], FP, tag="ds",
                                               name=f"ds_{t}")
                        cand_ring[t] = ring.tile([P, NCH * 8], FP, tag="cand",
                                                 name=f"cand_{t}")
                    ds = ds_ring[t]
                    cand = cand_ring[t]
                    dq = kps.tile([P, 1024], FP, tag="knn", name=f"dq_{t}_{q}")
                    for h in range(2):
                        c0 = h * 512
                        nc.tensor.matmul(dq[:, c0:c0 + 512],
                                         lhsT=ka[:, t * P:(t + 1) * P],
                                         rhs=kb[:, q * 1024 + c0:q * 1024 + c0 + 512],
                                         start=True, stop=True)
                    nc.vector.scalar_tensor_tensor(
                        out=ds[:, q * 1024:(q + 1) * 1024].bitcast(U32),
                        in0=dq[:, :].bitcast(U32), scalar=msk[:, :],
                        in1=iota[:, q * 1024:(q + 1) * 1024],
                        op0=AX.bitwise_and, op1=AX.bitwise_or)
                    if q == t // 8:
                        # self-distance kill lives in this quarter
                        nc.vector.tensor_tensor(out=ds[:, t * P:(t + 1) * P],
                                                in0=ds[:, t * P:(t + 1) * P],
                                                in1=ibig[:, :], op=AX.subtract)
                    for ch in (2 * q, 2 * q + 1):
                        nc.vector.max(cand[:, ch * 8:(ch + 1) * 8],
                                      ds[:, ch * CHW:(ch + 1) * CHW])

                def knn_select(s):
                    """Top-16 of the chunk candidates, index extract, gather."""
                    ds_ring.pop(s)
                    cand = cand_ring.pop(s)
                    nc.vector.max(t12[:, 0:8], cand[:, :])
                    nc.vector.match_replace(out=cand[:, :],
                                            in_to_replace=t12[:, 0:8],
                                            in_values=cand[:, :],
                                            imm_value=-1.0e30)
                    nc.vector.max(t12[:, 8:16], cand[:, :])
                    nc.vector.tensor_scalar(out=ix[:, :],
                                            in0=t12[:, :].bitcast(U32),
                                            scalar1=msk12[:, :], scalar2=None,
                                            op0=AX.bitwise_and)
                    idxJ = ring.tile([P, P], I16, tag="idxJ", bufs=4)
                    nc.vector.tensor_copy(
                        idxJ[:, 0:16].bitcast(U16),
                        ix[:, :].bitcast(U16)
                        .rearrange("p (k two) -> p k two", two=2)[:, :, 0])
                    nc.vector.tensor_copy(idxJ[:, 16:32], idxJ[:, 0:16])
                    nc.vector.tensor_copy(idxJ[:, 32:64], idxJ[:, 0:32])
                    nc.vector.tensor_copy(idxJ[:, 64:128], idxJ[:, 0:64])
                    idxT = ring.tile([P, P], I16, tag="idxT", bufs=4)
                    nc.sync.dma_start_transpose(idxT[:, :], idxJ[:, :])
                    a1g = ring.tile([P, K * P], BF, tag="a1g", bufs=3)
                    a1gv = a1g[:, :].rearrange("p (b c) -> p b c", b=K)
                    for g in range(2):
                        nc.gpsimd.dma_gather(
                            out_ap=a1gv[:, g * 8:(g + 1) * 8, :],
                            in_ap=a1bf[:, :],
                            idxs_ap=idxT[:, g * 64:(g + 1) * 64],
                            num_idxs=1024, num_idxs_reg=1024, elem_size=P,
                            transpose=False)
                    return a1g

                def fc_stages(u, a1g):
                    """Generator: one FC tile in 5 stages (yield between them)."""
                    co = u * 256
                    hstack = ring.tile([P, E], BF, tag="hstack")
                    h34 = ring.tile([P, E], BF, tag="h34")
                    scr = ring.tile([P, 2 * E], BF, tag="scr")
                    msbA = ring.tile([P, P], FP, tag="msbA")
                    msbB = ring.tile([P, P], FP, tag="msbB")
                    psf = fps.tile([P, E], FP, tag="fc", name=f"psf_{u}")

                    # --- stage 1: joint c1|c2 broadcast (start) then the
                    #     a1g transposes accumulate layer 1 in psf[0:64];
                    #     psf[64:128] holds c2 awaiting layer 2.
                    a1gv = a1g[:, :].rearrange("p (b c) -> p b c", b=K)
                    for n in range(4):
                        nc.tensor.matmul(psf[:, n * 512:(n + 1) * 512],
                                         lhsT=ctab[:, co:co + 2 * C],
                                         rhs=sp[:, n * 512:(n + 1) * 512],
                                         start=True, stop=False)
                    for bb in range(K):
                        nc.tensor.matmul(psf[0:C, bb * P:(bb + 1) * P],
                                         lhsT=a1gv[:, bb, 0:C],
                                         rhs=identfb[:, :],
                                         start=False, stop=(bb % 4 == 3))
                    for hh in range(2):
                        nc.scalar.activation(
                            hstack[0:C, hh * 1024:(hh + 1) * 1024],
                            psf[0:C, hh * 1024:(hh + 1) * 1024], RELU,
                            bias=b12[0:C, :], scale=1.0)
                    yield
                    # --- stage 2: layer 2 accumulates onto c2 in psf[64:128]
                    for n in range(4):
                        nc.tensor.matmul(psf[C:P, n * 512:(n + 1) * 512],
                                         lhsT=w2a[:, :],
                                         rhs=hstack[0:C, n * 512:(n + 1) * 512],
                                         start=False, stop=True)
                    for hh in range(2):
                        nc.scalar.activation(
                            hstack[C:P, hh * 1024:(hh + 1) * 1024],
                            psf[C:P, hh * 1024:(hh + 1) * 1024], RELU,
                            bias=b12[C:P, :], scale=1.0)
                    yield
                    # --- stage 3: layer 3 -> psf[0:64]; h1|h2 fold
                    for n in range(4):
                        nc.tensor.matmul(psf[0:C, n * 512:(n + 1) * 512],
                                         lhsT=w3ab[:, :],
                                         rhs=hstack[:, n * 512:(n + 1) * 512],
                                         start=True, stop=False)
                    for n in range(4):
                        nc.tensor.matmul(psf[0:C, n * 512:(n + 1) * 512],
                                         lhsT=ctab[:, co + 2 * C:co + 3 * C],
                                         rhs=sp[:, n * 512:(n + 1) * 512],
                                         start=False, stop=True)
                    _fold(nc, scr, hstack, msbB[:, :], P, 0)
                    for hh in range(2):
                        nc.scalar.activation(
                            h34[0:C, hh * 1024:(hh + 1) * 1024],
                            psf[0:C, hh * 1024:(hh + 1) * 1024], RELU,
                            bias=b43[0:C, :], scale=1.0)
                    yield
                    # --- stage 4: layer 4 -> psf[64:128]
                    for n in range(4):
                        nc.tensor.matmul(psf[C:P, n * 512:(n + 1) * 512],
                                         lhsT=w4a[:, :],
                                         rhs=h34[0:C, n * 512:(n + 1) * 512],
                                         start=True, stop=False)
                    for n in range(4):
                        nc.tensor.matmul(psf[C:P, n * 512:(n + 1) * 512],
                                         lhsT=w4bc[:, :],
                                         rhs=hstack[:, n * 512:(n + 1) * 512],
                                         start=False, stop=True)
                    for hh in range(2):
                        nc.scalar.copy(h34[C:P, hh * 1024:(hh + 1) * 1024],
                                       psf[C:P, hh * 1024:(hh + 1) * 1024])
                    yield
                    # --- stage 5: h3|h4 fold, post-max relu, transpose, out
                    _fold(nc, scr, h34, msbA[:, :], P, E)
                    nc.scalar.activation(msbB[:, :], msbB[:, :], RELU,
                                         bias=b12[:, :], scale=1.0)
                    nc.scalar.activation(msbA[0:C, :], msbA[0:C, :], RELU,
                                         bias=b43[0:C, :], scale=1.0)
                    pso = pps.tile([P, 2 * P], FP, tag="pso", name=f"pso_{u}")
                    nc.tensor.matmul(pso[:, 0:P], lhsT=msbA[:, :],
                                     rhs=identf[:, :], is_transpose=True,
                                     start=True, stop=False)
                    nc.tensor.matmul(pso[:, P:2 * P], lhsT=msbB[:, :],
                                     rhs=identf[:, :], is_transpose=True,
                                     start=False, stop=True)
                    outsb = ring.tile([P, D + 4 * C], FP, tag="outsb")
                    # order: [h4, h3, h2, h1, x]; msbA=[h3;h4], msbB=[h1;h2]
                    nc.scalar.copy(outsb[:, 0:C], pso[:, C:2 * C])
                    nc.scalar.copy(outsb[:, C:2 * C], pso[:, 0:C])
                    nc.scalar.copy(outsb[:, 2 * C:3 * C], pso[:, 3 * C:4 * C])
                    nc.scalar.copy(outsb[:, 3 * C:4 * C], pso[:, 2 * C:3 * C])
                    nc.vector.tensor_tensor(
                        out=outsb[:, 0:C], in0=outsb[:, 0:C],
                        in1=ctab[:, co + 3 * C:co + 4 * C], op=AX.add)
                    yield
                    # --- stage 6 (next iteration): output DMAs, by which
                    #     time the stage-5 compute has long finished, so the
                    #     SP queue never stalls waiting on them.
                    nc.sync.dma_start(outsb[:, 4 * C:4 * C + D],
                                      x_d[u * P:(u + 1) * P, :])
                    nc.sync.dma_start(out_d[u * P:(u + 1) * P, :], outsb[:, :])
                    yield

                def adv(g):
                    if g is not None:
                        next(g, None)

                # KNN mms at t, selection+gather at t-1, FC at t-3.
                a1g_ring = {}
                fc_tail = {}
                for it in range(NT + 4):
                    t, s, u = it, it - 1, it - 3
                    gprev = fc_tail.pop(it - 1, None)
                    adv(gprev)                  # FC(u-1) output DMAs
                    g = fc_stages(u, a1g_ring.pop(u)) if 0 <= u < NT else None
                    if g is not None:
                        fc_tail[it] = g
                    if 0 <= s < NT:
                        a1g_ring[s] = knn_select(s)
                    adv(g)                      # FC(u) stage 1
                    if t < NT:
                        knn_mm_stage(t, 0)
                    adv(g)                      # FC(u) stage 2
                    if t < NT:
                        knn_mm_stage(t, 1)
                    adv(g)                      # FC(u) stage 3
                    if t < NT:
                        knn_mm_stage(t, 2)
                    adv(g)                      # FC(u) stage 4
                    if t < NT:
                        knn_mm_stage(t, 3)
                    adv(g)                      # FC(u) stage 5

    nc.compile()
    return nc


def host_prep(x, pos, W_first, b_first, W_mid1, b_mid1, W_mid2, b_mid2,
              W_last, b_last):
    """Host-side arrangement of per-core inputs (numpy, cheap O(N) work)."""
    import ml_dtypes
    f32 = np.float32
    bf = ml_dtypes.bfloat16
    x = np.asarray(x, f32)
    pos = np.asarray(pos, f32)
    Wf = np.asarray(W_first, f32)
    Wm1 = np.asarray(W_mid1, f32)
    Wm2 = np.asarray(W_mid2, f32)
    Wl = np.asarray(W_last, f32)

    V1 = Wf[D:2 * D] + Wf[2 * D:3 * D]
    U1 = Wf[0:D] - Wf[2 * D:3 * D]
    W2a, W2x = Wm1[0:C], Wm1[C:C + D]
    W3a, W3b, W3c = Wm2[0:C], Wm2[C:2 * C], Wm2[2 * C:2 * C + D]
    W4a, W4b, W4c, W4d = Wl[0:C], Wl[C:2 * C], Wl[2 * C:3 * C], Wl[3 * C:3 * C + D]

    n = x.shape[1]
    b1 = np.asarray(b_first, f32).reshape(C, 1)
    b2 = np.asarray(b_mid1, f32).reshape(C, 1)
    b3v = np.asarray(b_mid2, f32).reshape(C, 1)

    e_idx = np.arange(P * K)
    sp = (np.arange(P)[:, None] == (e_idx // K)[None, :]).astype(bf)

    shared = {
        "wcat": np.concatenate([V1, U1, W2x, W3c, W4d], axis=1).astype(bf),
        "w2a": W2a.astype(bf),
        # hstack rows: [h1 (0:64); h2 (64:128)]
        "w3ab": np.concatenate([W3b, W3a], axis=0).astype(bf),
        "w4a": W4a.astype(bf),
        "w4bc": np.concatenate([W4c, W4b], axis=0).astype(bf),
        "b12": np.concatenate([b1, b2], axis=0).copy(),
        "b43": np.concatenate([b3v, np.zeros_like(b3v)], axis=0).copy(),
        "b4rb": np.broadcast_to(np.asarray(b_last, f32).reshape(1, C),
                                (P, C)).astype(bf),
        "sp": np.ascontiguousarray(sp),
        "identfb": np.eye(P, dtype=f32).astype(bf),
        "identf": np.eye(P, dtype=f32),
        "ibig": (np.eye(P, dtype=f32) * 1.0e38),
        "iota": np.broadcast_to(np.arange(n, dtype=np.uint32), (P, n)).copy(),
    }

    in_maps = []
    for bi in range(x.shape[0]):
        pb = pos[bi]                                  # (N, 3)
        sq = (pb * pb).sum(axis=-1, dtype=f32)        # (N,)
        ph = pb.astype(bf)
        pl = (pb - ph.astype(f32)).astype(bf)
        sqh = sq.astype(bf)
        sql = (sq - sqh.astype(f32)).astype(bf)
        ones = np.ones(n, f32)

        ka = np.zeros((16, n), f32)
        kb = np.zeros((16, n), f32)
        ph32, pl32 = ph.astype(f32), pl.astype(f32)
        ka[0:3] = 2.0 * ph32.T
        kb[0:3] = ph32.T
        ka[3:6] = 2.0 * ph32.T
        kb[3:6] = pl32.T
        ka[6:9] = 2.0 * pl32.T
        kb[6:9] = ph32.T
        ka[9] = sqh.astype(f32)
        kb[9] = -ones
        ka[10] = sql.astype(f32)
        kb[10] = -ones
        ka[11] = -ones
        kb[11] = sqh.astype(f32)
        ka[12] = -ones
        kb[12] = sql.astype(f32)

        m = dict(shared)
        m["x"] = np.ascontiguousarray(x[bi])
        m["xt"] = np.ascontiguousarray(x[bi].T).astype(bf)
        m["ka"] = ka.astype(bf)
        m["kb"] = kb.astype(bf)
        in_maps.append(m)
    return in_maps


_NC_CACHE = {}
LAST_RESULT = None


def kernel(**inputs):
    import os

    from concourse.bass_utils import run_bass_kernel_spmd

    global LAST_RESULT
    in_maps = host_prep(**inputs)
    n = inputs["x"].shape[1]
    if n not in _NC_CACHE:
        _NC_CACHE[n] = build_kernel(n)
    nc = _NC_CACHE[n]
    trace = bool(os.environ.get("KERNEL_TRACE"))
    res = run_bass_kernel_spmd(nc, in_maps, core_ids=list(range(len(in_maps))),
                               trace=trace)
    LAST_RESULT = res
    out = np.stack([r["out"] for r in res.results], axis=0)
    return out


# revision 17
# speedup vs baseline: 1.5027x; 1.0626x over previous
# DenseEdgeConv (B=8, N=4096, D=128, K=16, C=64) Trainium2 Bass kernel, v3.
#
# Data-parallel over B (one point cloud per core). Per core:
#
#   KNN:  ds = -d2 computed on the PE as a 13-row bf16 hi/lo-split matmul
#         (products hi*hi + hi*lo + lo*hi; sq_i/sq_j split hi+lo; error
#         ~2^-17, far below the rank-16/17 distance gap). The DVE then
#         "twiddles" ds: low 12 mantissa bits := column index j (one
#         scalar_tensor_tensor pass, PSUM->SBUF). Ordering keeps 11
#         mantissa bits -- ample for neighbor selection. Top-16 per row
#         via 8 per-512-chunk max8 calls + top-16 of the 64 candidates;
#         neighbor indices fall out of the low bits of the winners (no
#         full-row max_index passes).
#   Gather: indices go to the [16,128]-replicated layout with one XBAR
#         dma transpose, then 4 batched transpose dma_gathers pull the
#         2048 neighbor rows of the bf16 a1-table directly into the
#         transposed [64ch, 2048edge] layout.
#   FC:   algebraic restructure (per-point tables a1/c1..c4):
#           h1 = relu(a1[j] + c1[i] + b1)
#           h2 = relu(W2a^T h1 + c2[i] + b2)
#           h3 = relu(W3a^T h2 + W3b^T h1 + c3[i] + b3)
#           h4 = W4a^T h3 + W4b^T h2 + W4c^T h1   (+ c4[i] + b4 post-max)
#         All matmuls bf16 (1 cycle/row); c-terms broadcast over k with
#         S' = I128 (x) ones(16) selection matmuls. Edge order e = i*16+k
#         (k innermost) so max over k is a contiguous fold, done as bf16
#         tensor-tensor max folds. relu/bias commute with the max
#         (monotone, per-point constant). h1/h2 share one [128, E] tile
#         and h3/h4 another, so each fold covers two layers.
#   out[i] = [h4max + c4 + b4, h3max, h2max, h1max, x[i]]
#
# The per-tile work is software-pipelined: FC lags KNN by 2 tiles and FC
# stages are emitted interleaved with the KNN matmul quarters so no
# engine queue head-blocks.

import numpy as np

import concourse.bacc as bacc
import concourse.bass as bass
import concourse.mybir as mybir
import concourse.tile as tile

FP = mybir.dt.float32
BF = mybir.dt.bfloat16
U32 = mybir.dt.uint32
U16 = mybir.dt.uint16
I16 = mybir.dt.int16

B, N_FULL, D, K, C = 8, 4096, 128, 16, 64
P = 128
E = P * K
AX = mybir.AluOpType
RELU = mybir.ActivationFunctionType.Relu


def _fold(nc, scr, h, out_ap, rows, s0):
    """max over k=16 (innermost, contiguous) of h [rows, (i k)] -> out [rows, 128]."""
    v = h[0:rows, :].rearrange("c (i k) -> c i k", k=16)
    f1 = scr[0:rows, s0:s0 + E // 2].rearrange("c (i k) -> c i k", k=8)
    nc.vector.tensor_tensor(out=f1, in0=v[:, :, 0:8], in1=v[:, :, 8:16], op=AX.max)
    f2 = scr[0:rows, s0 + E // 2:s0 + 3 * E // 4].rearrange("c (i k) -> c i k", k=4)
    nc.vector.tensor_tensor(out=f2, in0=f1[:, :, 0:4], in1=f1[:, :, 4:8], op=AX.max)
    f3 = scr[0:rows, s0 + 3 * E // 4:s0 + 7 * E // 8].rearrange("c (i k) -> c i k", k=2)
    nc.vector.tensor_tensor(out=f3, in0=f2[:, :, 0:2], in1=f2[:, :, 2:4], op=AX.max)
    nc.vector.tensor_tensor(out=out_ap, in0=f3[:, :, 0], in1=f3[:, :, 1], op=AX.max)


def build_kernel(N=N_FULL):
    NT = N // P          # 32 point tiles
    NCH = 8              # knn max8 chunks per row
    CHW = N // NCH       # 512

    nc = bacc.Bacc("TRN2", target_bir_lowering=False, debug=False)

    x_d = nc.dram_tensor("x", [N, D], FP, kind="ExternalInput").ap()
    xt_d = nc.dram_tensor("xt", [D, N], BF, kind="ExternalInput").ap()
    ka_d = nc.dram_tensor("ka", [16, N], BF, kind="ExternalInput").ap()
    kb_d = nc.dram_tensor("kb", [16, N], BF, kind="ExternalInput").ap()
    wcat_d = nc.dram_tensor("wcat", [D, 320], BF, kind="ExternalInput").ap()
    w2a_d = nc.dram_tensor("w2a", [C, C], BF, kind="ExternalInput").ap()
    w3ab_d = nc.dram_tensor("w3ab", [2 * C, C], BF, kind="ExternalInput").ap()
    w4a_d = nc.dram_tensor("w4a", [C, C], BF, kind="ExternalInput").ap()
    w4bc_d = nc.dram_tensor("w4bc", [2 * C, C], BF, kind="ExternalInput").ap()
    b12_d = nc.dram_tensor("b12", [P, 1], FP, kind="ExternalInput").ap()
    b43_d = nc.dram_tensor("b43", [P, 1], FP, kind="ExternalInput").ap()
    b4rb_d = nc.dram_tensor("b4rb", [P, C], BF, kind="ExternalInput").ap()
    sp_d = nc.dram_tensor("sp", [P, E], BF, kind="ExternalInput").ap()
    identfb_d = nc.dram_tensor("identfb", [P, P], BF, kind="ExternalInput").ap()
    identf_d = nc.dram_tensor("identf", [P, P], FP, kind="ExternalInput").ap()
    ibig_d = nc.dram_tensor("ibig", [P, P], FP, kind="ExternalInput").ap()
    iota_d = nc.dram_tensor("iota", [P, N], U32, kind="ExternalInput").ap()

    out_d = nc.dram_tensor("out", [N, D + 4 * C], FP, kind="ExternalOutput").ap()

    with tile.TileContext(nc) as tc:
        with (
            tc.tile_pool(name="const", bufs=1) as cpool,
            tc.tile_pool(name="persist", bufs=1) as ppool,
            tc.tile_pool(name="dram", bufs=1, space="DRAM") as dpool,
        ):
            def cin(name, shape, dt, src):
                tl = cpool.tile(shape, dt, name=name)
                nc.sync.dma_start(tl[:, :], src)
                return tl

            ka = cin("ka", [16, N], BF, ka_d)
            kb = cin("kb", [16, N], BF, kb_d)
            wcat = cin("wcat", [D, 320], BF, wcat_d)
            w2a = cin("w2a", [C, C], BF, w2a_d)
            w3ab = cin("w3ab", [2 * C, C], BF, w3ab_d)
            w4a = cin("w4a", [C, C], BF, w4a_d)
            w4bc = cin("w4bc", [2 * C, C], BF, w4bc_d)
            b12 = cin("b12", [P, 1], FP, b12_d)
            b43 = cin("b43", [P, 1], FP, b43_d)
            b4rb = cin("b4rb", [P, C], BF, b4rb_d)
            sp = cin("sp", [P, E], BF, sp_d)
            identfb = cin("identfb", [P, P], BF, identfb_d)
            identf = cin("identf", [P, P], FP, identf_d)
            ibig = cin("ibig", [P, P], FP, ibig_d)
            iota = cin("iota", [P, N], U32, iota_d)
            msk = cpool.tile([P, 1], U32)
            nc.gpsimd.memset(msk[:, :], int(0xFFFFF000))
            msk12 = cpool.tile([P, 1], U32)
            nc.gpsimd.memset(msk12[:, :], int(0xFFF))

            ctab = ppool.tile([P, NT * 256], BF)   # [c1|c2|c3|c4] per tile
            a1bf = dpool.tile([N, P], BF)          # a1 rows padded to 256B

            # ============ Phase A: per-point tables ============
            with (
                tc.tile_pool(name="axt", bufs=1) as axt,
                tc.tile_pool(name="apsum", bufs=2, space="PSUM") as aps,
                tc.tile_pool(name="asb", bufs=2) as asb,
            ):
                xt = axt.tile([D, N], BF)
                nc.sync.dma_start(xt[:, :], xt_d)
                for t in range(NT):
                    cps = aps.tile([P, 320], FP, tag="cps")
                    nc.tensor.matmul(cps[:, :], lhsT=xt[:, t * P:(t + 1) * P],
                                     rhs=wcat[:, :], start=True, stop=True)
                    a1s = asb.tile([P, P], BF, tag="a1s")
                    nc.gpsimd.memset(a1s[:, C:P], 0.0)
                    nc.scalar.copy(a1s[:, 0:C], cps[:, 0:C])
                    nc.sync.dma_start(a1bf[t * P:(t + 1) * P, :], a1s[:, :])
                    nc.scalar.copy(ctab[:, t * 256:(t + 1) * 256], cps[:, C:320])
                    nc.vector.tensor_tensor(
                        out=ctab[:, t * 256 + 192:t * 256 + 256],
                        in0=ctab[:, t * 256 + 192:t * 256 + 256],
                        in1=b4rb[:, :], op=AX.add)

            # ============ Phase B: fused KNN + FC, software-pipelined ======
            with (
                tc.tile_pool(name="kps", bufs=1, space="PSUM") as kps,
                tc.tile_pool(name="pps", bufs=2, space="PSUM") as pps,
                tc.tile_pool(name="fps", bufs=1, space="PSUM") as fps,
                tc.tile_pool(name="knnsb", bufs=1) as ksb,
                tc.tile_pool(name="ring", bufs=2) as ring,
            ):
                t12 = ksb.tile([P, 16], FP)
                ix = ksb.tile([P, 16], U32)
                ds_ring = {}
                cand_ring = {}

                def knn_mm_stage(t, q):
                    """One PSUM quarter of distances, twiddle, chunk maxes."""
                    if q == 0:
                        ds_ring[t] = ring.tile([P, N], FP, tag="ds",
                                               name=f"ds_{t}")
                        cand_ring[t] = ring.tile([P, NCH * 8], FP, tag="cand",
                                                 name=f"cand_{t}")
                    ds = ds_ring[t]
                    cand = cand_ring[t]
                    dq = kps.tile([P, 1024], FP, tag="knn", name=f"dq_{t}_{q}")
                    for h in range(2):
                        c0 = h * 512
                        nc.tensor.matmul(dq[:, c0:c0 + 512],
                                         lhsT=ka[:, t * P:(t + 1) * P],
                                         rhs=kb[:, q * 1024 + c0:q * 1024 + c0 + 512],
                                         start=True, stop=True)
                    nc.vector.scalar_tensor_tensor(
                        out=ds[:, q * 1024:(q + 1) * 1024].bitcast(U32),
                        in0=dq[:, :].bitcast(U32), scalar=msk[:, :],
                        in1=iota[:, q * 1024:(q + 1) * 1024],
                        op0=AX.bitwise_and, op1=AX.bitwise_or)
                    if q == t // 8:
                        # self-distance kill lives in this quarter
                        nc.vector.tensor_tensor(out=ds[:, t * P:(t + 1) * P],
                                                in0=ds[:, t * P:(t + 1) * P],
                                                in1=ibig[:, :], op=AX.subtract)
                    for ch in (2 * q, 2 * q + 1):
                        nc.vector.max(cand[:, ch * 8:(ch + 1) * 8],
                                      ds[:, ch * CHW:(ch + 1) * CHW])

                def knn_select(s):
                    """Top-16 of the chunk candidates, index extract, gather."""
                    ds_ring.pop(s)
                    cand = cand_ring.pop(s)
                    nc.vector.max(t12[:, 0:8], cand[:, :])
                    nc.vector.match_replace(out=cand[:, :],
                                            in_to_replace=t12[:, 0:8],
                                            in_values=cand[:, :],
                                            imm_value=-1.0e30)
                    nc.vector.max(t12[:, 8:16], cand[:, :])
                    nc.vector.tensor_scalar(out=ix[:, :],
                                            in0=t12[:, :].bitcast(U32),
                                            scalar1=msk12[:, :], scalar2=None,
                                            op0=AX.bitwise_and)
                    idxJ = ring.tile([P, P], I16, tag="idxJ", bufs=4)
                    nc.vector.tensor_copy(
                        idxJ[:, 0:16].bitcast(U16),
                        ix[:, :].bitcast(U16)
                        .rearrange("p (k two) -> p k two", two=2)[:, :, 0])
                    nc.gpsimd.tensor_copy(idxJ[:, 16:32], idxJ[:, 0:16])
                    nc.gpsimd.tensor_copy(idxJ[:, 32:64], idxJ[:, 0:32])
                    nc.gpsimd.tensor_copy(idxJ[:, 64:128], idxJ[:, 0:64])
                    idxT = ring.tile([P, P], I16, tag="idxT", bufs=4)
                    nc.sync.dma_start_transpose(idxT[:, :], idxJ[:, :])
                    a1g = ring.tile([P, K * P], BF, tag="a1g", bufs=3)
                    a1gv = a1g[:, :].rearrange("p (b c) -> p b c", b=K)
                    for g in range(2):
                        nc.gpsimd.dma_gather(
                            out_ap=a1gv[:, g * 8:(g + 1) * 8, :],
                            in_ap=a1bf[:, :],
                            idxs_ap=idxT[:, g * 64:(g + 1) * 64],
                            num_idxs=1024, num_idxs_reg=1024, elem_size=P,
                            transpose=False)
                    return a1g

                def fc_stages(u, a1g):
                    """Generator: one FC tile in 5 stages (yield between them)."""
                    co = u * 256
                    hstack = ring.tile([P, E], BF, tag="hstack")
                    h34 = ring.tile([P, E], BF, tag="h34")
                    scr = ring.tile([P, 2 * E], BF, tag="scr")
                    msbA = ring.tile([P, P], FP, tag="msbA")
                    msbB = ring.tile([P, P], FP, tag="msbB")
                    psf = fps.tile([P, E], FP, tag="fc", name=f"psf_{u}")

                    # --- stage 1: joint c1|c2 broadcast (start) then the
                    #     a1g transposes accumulate layer 1 in psf[0:64];
                    #     psf[64:128] holds c2 awaiting layer 2.
                    a1gv = a1g[:, :].rearrange("p (b c) -> p b c", b=K)
                    for n in range(4):
                        nc.tensor.matmul(psf[:, n * 512:(n + 1) * 512],
                                         lhsT=ctab[:, co:co + 2 * C],
                                         rhs=sp[:, n * 512:(n + 1) * 512],
                                         start=True, stop=False)
                    for bb in range(K):
                        nc.tensor.matmul(psf[0:C, bb * P:(bb + 1) * P],
                                         lhsT=a1gv[:, bb, 0:C],
                                         rhs=identfb[:, :],
                                         start=False, stop=(bb % 4 == 3))
                    for hh in range(2):
                        nc.scalar.activation(
                            hstack[0:C, hh * 1024:(hh + 1) * 1024],
                            psf[0:C, hh * 1024:(hh + 1) * 1024], RELU,
                            bias=b12[0:C, :], scale=1.0)
                    yield
                    # --- stage 2: layer 2 accumulates onto c2 in psf[64:128]
                    for n in range(4):
                        nc.tensor.matmul(psf[C:P, n * 512:(n + 1) * 512],
                                         lhsT=w2a[:, :],
                                         rhs=hstack[0:C, n * 512:(n + 1) * 512],
                                         start=False, stop=True)
                    for hh in range(2):
                        nc.scalar.activation(
                            hstack[C:P, hh * 1024:(hh + 1) * 1024],
                            psf[C:P, hh * 1024:(hh + 1) * 1024], RELU,
                            bias=b12[C:P, :], scale=1.0)
                    yield
                    # --- stage 3: layer 3 -> psf[0:64]; h1|h2 fold
                    for n in range(4):
                        nc.tensor.matmul(psf[0:C, n * 512:(n + 1) * 512],
                                         lhsT=w3ab[:, :],
                                         rhs=hstack[:, n * 512:(n + 1) * 512],
                                         start=True, stop=False)
                    for n in range(4):
                        nc.tensor.matmul(psf[0:C, n * 512:(n + 1) * 512],
                                         lhsT=ctab[:, co + 2 * C:co + 3 * C],
                                         rhs=sp[:, n * 512:(n + 1) * 512],
                                         start=False, stop=True)
                    _fold(nc, scr, hstack, msbB[:, :], P, 0)
                    for hh in range(2):
                        nc.scalar.activation(
                            h34[0:C, hh * 1024:(hh + 1) * 1024],
                            psf[0:C, hh * 1024:(hh + 1) * 1024], RELU,
                            bias=b43[0:C, :], scale=1.0)
                    yield
                    # --- stage 4: layer 4 -> psf[64:128]
                    for n in range(4):
                        nc.tensor.matmul(psf[C:P, n * 512:(n + 1) * 512],
                                         lhsT=w4a[:, :],
                                         rhs=h34[0:C, n * 512:(n + 1) * 512],
                                         start=True, stop=False)
                    for n in range(4):
                        nc.tensor.matmul(psf[C:P, n * 512:(n + 1) * 512],
                                         lhsT=w4bc[:, :],
                                         rhs=hstack[:, n * 512:(n + 1) * 512],
                                         start=False, stop=True)
                    for hh in range(2):
                        nc.scalar.copy(h34[C:P, hh * 1024:(hh + 1) * 1024],
                                       psf[C:P, hh * 1024:(hh + 1) * 1024])
                    yield
                    # --- stage 5: h3|h4 fold, post-max relu, transpose, out
                    _fold(nc, scr, h34, msbA[:, :], P, E)
                    nc.scalar.activation(msbB[:, :], msbB[:, :], RELU,
                                         bias=b12[:, :], scale=1.0)
                    nc.scalar.activation(msbA[0:C, :], msbA[0:C, :], RELU,
                                         bias=b43[0:C, :], scale=1.0)
                    pso = pps.tile([P, 2 * P], FP, tag="pso", name=f"pso_{u}")
                    nc.tensor.matmul(pso[:, 0:P], lhsT=msbA[:, :],
                                     rhs=identf[:, :], is_transpose=True,
                                     start=True, stop=False)
                    nc.tensor.matmul(pso[:, P:2 * P], lhsT=msbB[:, :],
                                     rhs=identf[:, :], is_transpose=True,
                                     start=False, stop=True)
                    outsb = ring.tile([P, D + 4 * C], FP, tag="outsb")
                    # order: [h4, h3, h2, h1, x]; msbA=[h3;h4], msbB=[h1;h2]
                    nc.scalar.copy(outsb[:, 0:C], pso[:, C:2 * C])
                    nc.scalar.copy(outsb[:, C:2 * C], pso[:, 0:C])
                    nc.scalar.copy(outsb[:, 2 * C:3 * C], pso[:, 3 * C:4 * C])
                    nc.scalar.copy(outsb[:, 3 * C:4 * C], pso[:, 2 * C:3 * C])
                    nc.vector.tensor_tensor(
                        out=outsb[:, 0:C], in0=outsb[:, 0:C],
                        in1=ctab[:, co + 3 * C:co + 4 * C], op=AX.add)
                    yield
                    # --- stage 6 (next iteration): output DMAs, by which
                    #     time the stage-5 compute has long finished, so the
                    #     SP queue never stalls waiting on them.
                    nc.sync.dma_start(outsb[:, 4 * C:4 * C + D],
                                      x_d[u * P:(u + 1) * P, :])
                    nc.sync.dma_start(out_d[u * P:(u + 1) * P, :], outsb[:, :])
                    yield

                def adv(g):
                    if g is not None:
                        next(g, None)

                # KNN mms at t, selection+gather at t-1, FC at t-3.
                a1g_ring = {}
                fc_tail = {}
                for it in range(NT + 4):
                    t, s, u = it, it - 1, it - 3
                    gprev = fc_tail.pop(it - 1, None)
                    adv(gprev)                  # FC(u-1) output DMAs
                    g = fc_stages(u, a1g_ring.pop(u)) if 0 <= u < NT else None
                    if g is not None:
                        fc_tail[it] = g
                    if 0 <= s < NT:
                        a1g_ring[s] = knn_select(s)
                    adv(g)                      # FC(u) stage 1
                    if t < NT:
                        knn_mm_stage(t, 0)
                    adv(g)                      # FC(u) stage 2
                    if t < NT:
                        knn_mm_stage(t, 1)
                    adv(g)                      # FC(u) stage 3
                    if t < NT:
                        knn_mm_stage(t, 2)
                    adv(g)                      # FC(u) stage 4
                    if t < NT:
                        knn_mm_stage(t, 3)
                    adv(g)                      # FC(u) stage 5

    nc.compile()
    return nc


def host_prep(x, pos, W_first, b_first, W_mid1, b_mid1, W_mid2, b_mid2,
              W_last, b_last):
    """Host-side arrangement of per-core inputs (numpy, cheap O(N) work)."""
    import ml_dtypes
    f32 = np.float32
    bf = ml_dtypes.bfloat16
    x = np.asarray(x, f32)
    pos = np.asarray(pos, f32)
    Wf = np.asarray(W_first, f32)
    Wm1 = np.asarray(W_mid1, f32)
    Wm2 = np.asarray(W_mid2, f32)
    Wl = np.asarray(W_last, f32)

    V1 = Wf[D:2 * D] + Wf[2 * D:3 * D]
    U1 = Wf[0:D] - Wf[2 * D:3 * D]
    W2a, W2x = Wm1[0:C], Wm1[C:C + D]
    W3a, W3b, W3c = Wm2[0:C], Wm2[C:2 * C], Wm2[2 * C:2 * C + D]
    W4a, W4b, W4c, W4d = Wl[0:C], Wl[C:2 * C], Wl[2 * C:3 * C], Wl[3 * C:3 * C + D]

    n = x.shape[1]
    b1 = np.asarray(b_first, f32).reshape(C, 1)
    b2 = np.asarray(b_mid1, f32).reshape(C, 1)
    b3v = np.asarray(b_mid2, f32).reshape(C, 1)

    e_idx = np.arange(P * K)
    sp = (np.arange(P)[:, None] == (e_idx // K)[None, :]).astype(bf)

    shared = {
        "wcat": np.concatenate([V1, U1, W2x, W3c, W4d], axis=1).astype(bf),
        "w2a": W2a.astype(bf),
        # hstack rows: [h1 (0:64); h2 (64:128)]
        "w3ab": np.concatenate([W3b, W3a], axis=0).astype(bf),
        "w4a": W4a.astype(bf),
        "w4bc": np.concatenate([W4c, W4b], axis=0).astype(bf),
        "b12": np.concatenate([b1, b2], axis=0).copy(),
        "b43": np.concatenate([b3v, np.zeros_like(b3v)], axis=0).copy(),
        "b4rb": np.broadcast_to(np.asarray(b_last, f32).reshape(1, C),
                                (P, C)).astype(bf),
        "sp": np.ascontiguousarray(sp),
        "identfb": np.eye(P, dtype=f32).astype(bf),
        "identf": np.eye(P, dtype=f32),
        "ibig": (np.eye(P, dtype=f32) * 1.0e38),
        "iota": np.broadcast_to(np.arange(n, dtype=np.uint32), (P, n)).copy(),
    }

    in_maps = []
    for bi in range(x.shape[0]):
        pb = pos[bi]                                  # (N, 3)
        sq = (pb * pb).sum(axis=-1, dtype=f32)        # (N,)
        ph = pb.astype(bf)
        pl = (pb - ph.astype(f32)).astype(bf)
        sqh = sq.astype(bf)
        sql = (sq - sqh.astype(f32)).astype(bf)
        ones = np.ones(n, f32)

        ka = np.zeros((16, n), f32)
        kb = np.zeros((16, n), f32)
        ph32, pl32 = ph.astype(f32), pl.astype(f32)
        ka[0:3] = 2.0 * ph32.T
        kb[0:3] = ph32.T
        ka[3:6] = 2.0 * ph32.T
        kb[3:6] = pl32.T
        ka[6:9] = 2.0 * pl32.T
        kb[6:9] = ph32.T
        ka[9] = sqh.astype(f32)
        kb[9] = -ones
        ka[10] = sql.astype(f32)
        kb[10] = -ones
        ka[11] = -ones
        kb[11] = sqh.astype(f32)
        ka[12] = -ones
        kb[12] = sql.astype(f32)

        m = dict(shared)
        m["x"] = np.ascontiguousarray(x[bi])
        m["xt"] = np.ascontiguousarray(x[bi].T).astype(bf)
        m["ka"] = ka.astype(bf)
        m["kb"] = kb.astype(bf)
        in_maps.append(m)
    return in_maps


_NC_CACHE = {}
LAST_RESULT = None


def kernel(**inputs):
    import os

    from concourse.bass_utils import run_bass_kernel_spmd

    global LAST_RESULT
    in_maps = host_prep(**inputs)
    n = inputs["x"].shape[1]
    if n not in _NC_CACHE:
        _NC_CACHE[n] = build_kernel(n)
    nc = _NC_CACHE[n]
    trace = bool(os.environ.get("KERNEL_TRACE"))
    res = run_bass_kernel_spmd(nc, in_maps, core_ids=list(range(len(in_maps))),
                               trace=trace)
    LAST_RESULT = res
    out = np.stack([r["out"] for r in res.results], axis=0)
    return out
